# revision 30
# baseline (speedup 1.0000x reference)
"""Trainium2 Bass kernel for nn_AutoregressiveLSA — fp8 DoubleRow version.

Math (complex, per batch b, one NeuronCore per batch element):
    Q  = WKQ @ E                       [2d, T]
    S  = E^H @ Q, keep i <= j          [T, T]
    outT[j] = sum_{i<=j} S[i,j] PT[i] * 2/max(j,1),  PT = (WPV @ E)^T

All matmuls run as fp8e4 (e4m3) in DoubleRow perf mode: one PE
instruction contracts TWO 128-chunks at 0.5 cycles/output-column (4x
the fp32r MAC rate).  Precision comes from a hi/lo split of every
operand (x ~ x_h + x_l, both e4m3; x_l*y_l dropped): per 128-chunk each
real product needs 3 fp8 pairings = 1.5 DR instructions, so a complex
Karatsuba product costs 2.25 free-columns/chunk vs 3.0 for fp32r.
Measured end-to-end rel err ~3e-3 (gate 2e-2).

Scale chain (powers of 2, folded into casts / final rho):
    E*4, WKQ^T*256, WPV^T*256 quantized on host.
    A1 psum = 1024*Q,  split scale 2^-7  -> Q'' = 8Q
    A2 psum = 1024*PT, split scale 2^-7  -> PT'' = 8PT
    B  psum = 32*S,    split scale 2^-9  -> S'' = S/16
    C  psum = S*PT/2,  rho2 = 2/max(j,1) applied via Act scale.

Engine constraints honored (probed on real TRN2): vector ops may read
at most ONE psum operand; Pool (gpsimd) runs SBUF-only tensor_tensor
(no psum, no scalar_tensor_tensor); Act does scaled copies (fp8 out ok).
Evacuation is fused into wide ops: psum banks ordered (M2, M1, M3) so
one 3W psum->sbuf copy + one dual-sub [re,tt] + pool im/sum + ONE 3W
Act h-cast + ONE 3W DVE stt l-split handle a whole complex site.
Phase B uses a host-negated E_im pack (nei) so its conjugated
recombination has the same (M1-M2', M3-M1-M2') form as the others.
"""

import numpy as np
import ml_dtypes

import concourse.bass as bass
import concourse.mybir as mybir
import concourse.tile as tile
from concourse import bacc
from concourse.bass_utils import run_bass_kernel_spmd
from concourse.alu_op_type import AluOpType

F32 = mybir.dt.float32
F8 = mybir.dt.float8e4
E4NP = ml_dtypes.float8_e4m3
DR = mybir.MatmulPerfMode.DoubleRow
COPY = mybir.ActivationFunctionType.Copy

B = 8
D2 = 1024
T = 2048
D = 512
P = 128
KC = D2 // P
MB = D2 // P
TB = T // P
A1W = 512
NJP = T // A1W
SPAN = 256
NSP = T // SPAN

CQ = float(2.0 ** -7)
CS = float(2.0 ** -9)


def pack_h0(t, fsl):
    """Slicer for h-first packs [P, K, 2(h,l), F] (E/S side)."""
    def f(k, kind):
        if kind == "hh":
            return t[:, 2 * k:2 * k + 2, 0, fsl]
        return t[:, k, :, fsl]
    return f


def pack_h1(t, fsl):
    """Slicer for l-first packs [P, K, 2(l,h), F] (W/Q/PT side)."""
    def f(k, kind):
        if kind == "hh":
            return t[:, 2 * k:2 * k + 2, 1, fsl]
        return t[:, k, :, fsl]
    return f


def dr_product(nc, bank, lhs, rhs, nk, leftover=None):
    nhh = nk // 2
    odd = nk % 2
    tot = nhh + nk + (1 if odd else 0)
    i = 0
    for kp in range(nhh):
        nc.tensor.matmul(bank, lhs(kp, "hh"), rhs(kp, "hh"),
                         start=(i == 0), stop=(i == tot - 1), perf_mode=DR)
        i += 1
    for k in range(nk):
        nc.tensor.matmul(bank, lhs(k, "x"), rhs(k, "x"),
                         start=(i == 0), stop=(i == tot - 1), perf_mode=DR)
        i += 1
    if odd:
        la, ra = leftover
        nc.tensor.matmul(bank, la, ra, start=(i == 0), stop=(i == tot - 1))


def build_module():
    nc = bacc.Bacc(target_bir_lowering=False, trn_type="TRN2")

    ep_r = nc.dram_tensor("ep_r", [P, KC, 2, T], F8, kind="ExternalInput")
    ep_i = nc.dram_tensor("ep_i", [P, KC, 2, T], F8, kind="ExternalInput")
    ep_ni = nc.dram_tensor("ep_ni", [P, KC, 2, T], F8, kind="ExternalInput")
    ep_s = nc.dram_tensor("ep_s", [P, KC, 2, T], F8, kind="ExternalInput")
    ep_d = nc.dram_tensor("ep_d", [P, KC, 2, T], F8, kind="ExternalInput")
    wp_all = nc.dram_tensor("wp_all", [MB, P, 3, KC, 2, P], F8,
                            kind="ExternalInput")
    vp_r = nc.dram_tensor("vp_r", [KC, P, 2, D], F8, kind="ExternalInput")
    vp_i = nc.dram_tensor("vp_i", [KC, P, 2, D], F8, kind="ExternalInput")
    vp_s = nc.dram_tensor("vp_s", [KC, P, 2, D], F8, kind="ExternalInput")
    trimask = nc.dram_tensor("trimask", [P, P], F32, kind="ExternalInput")
    rho2 = nc.dram_tensor("rho2", [P, TB], F32, kind="ExternalInput")
    outT_re = nc.dram_tensor("outT_re", [T, D], F32, kind="ExternalOutput")
    outT_im = nc.dram_tensor("outT_im", [T, D], F32, kind="ExternalOutput")

    _n = [0]

    def uid():
        _n[0] += 1
        return _n[0]

    with tile.TileContext(nc) as tc:
        with tc.tile_pool(name="dram", bufs=1, space="DRAM") as dram, \
             tc.tile_pool(name="erp", bufs=1) as erp, \
             tc.tile_pool(name="cst", bufs=1) as cst:
            q = dram.tile([MB, NSP, P, 6, SPAN], F8, tag="q")
            pt = dram.tile([TB, P, 6, D], F8, tag="pt")
            s = dram.tile([TB, TB, P, 6, P], F8, tag="s")

            er = erp.tile([P, KC, 2, T], F8, tag="er")
            mask_sb = cst.tile([P, P], F32, tag="mask")
            rho_sb = cst.tile([P, TB], F32, tag="rho")

            def site_evac(pp, width, c, pk_h_ap, pk_l_ap, ev_pool, rc_pool,
                          masks=None):
                """Evacuate one complex site.

                pp: psum tile [P, 3, width] with banks (M2, M1, M3).
                pk_h_ap/pk_l_ap: output APs for h/l fp8 splits of
                (re, im, sum), or None to skip splits (phase C).
                Returns ev tile [P, 4, width] = (re, im, sum, tt).
                """
                n = uid()
                rc = rc_pool.tile([P, 3, width], F32, tag="rc", name=f"rc{n}")
                ev = ev_pool.tile([P, 4, width], F32, tag="ev", name=f"ev{n}")
                nc.scalar.activation(rc[:], pp[:], COPY)
                nc.vector.tensor_sub(ev[:, 0::3], rc[:, 1:3], rc[:, 0:2])
                nc.gpsimd.tensor_sub(ev[:, 1], ev[:, 3], rc[:, 0])
                if masks is not None:
                    for dsl in masks:
                        nc.vector.tensor_mul(ev[:, 0, dsl], ev[:, 0, dsl],
                                             mask_sb[:])
                        nc.vector.tensor_mul(ev[:, 1, dsl], ev[:, 1, dsl],
                                             mask_sb[:])
                if pk_h_ap is None:
                    return ev
                nc.gpsimd.tensor_add(ev[:, 2], ev[:, 0], ev[:, 1])
                nc.scalar.activation(pk_h_ap, ev[:, 0:3], COPY, scale=c)
                nc.vector.scalar_tensor_tensor(
                    out=pk_l_ap, in0=ev[:, 0:3], scalar=c, in1=pk_h_ap,
                    op0=AluOpType.mult, op1=AluOpType.subtract)
                return ev

            # =============== Phases A1 + A2 (merged psum scope) ===========
            qsbp_cm = tc.tile_pool(name="qsbp", bufs=2)
            qsbp = qsbp_cm.__enter__()
            bd01_cm = tc.tile_pool(name="bd01", bufs=1)
            bd01 = bd01_cm.__enter__()
            nei01 = bd01.tile([P, KC, 2, T // 2], F8, tag="nei01")
            ed01 = bd01.tile([P, KC, 2, T // 2], F8, tag="ed01")
            qsb_tiles = {}

            def load_qsb(sp):
                t = qsbp.tile([P, MB, 6, SPAN], F8, tag="qsb",
                              name=f"qsb{sp}")
                nc.sync.dma_start(
                    t[:], q[:, sp].rearrange("m p v t -> p m v t"))
                qsb_tiles[sp] = t

            with tc.tile_pool(name="eip", bufs=1) as eip, \
                 tc.tile_pool(name="esp", bufs=1) as esp:
                # ei/es are rolling 2-panel windows (A2+A1 consume jp-wise)
                eiw = [eip.tile([P, KC, 2, A1W], F8, tag=f"eiw{h}",
                                name=f"eiw{h}") for h in range(2)]
                esw = [esp.tile([P, KC, 2, A1W], F8, tag=f"esw{h}",
                                name=f"esw{h}") for h in range(2)]

                with tc.tile_pool(name="psA", bufs=2, space="PSUM") as psA, \
                     tc.tile_pool(name="rcA", bufs=2) as rcA, \
                     tc.tile_pool(name="evA", bufs=2) as evA, \
                     tc.tile_pool(name="pkA", bufs=3) as pkA, \
                     tc.tile_pool(name="wroll", bufs=2) as wrollp, \
                     tc.tile_pool(name="vres", bufs=1) as vres:
                    vr = vres.tile([P, KC, 2, D], F8, tag="vr")
                    vi = vres.tile([P, KC, 2, D], F8, tag="vi")
                    vs = vres.tile([P, KC, 2, D], F8, tag="vs")

                    w_tiles = {}

                    def load_w(key, m):
                        n = uid()
                        wt = wrollp.tile([P, 3, KC, 2, P], F8, tag="wr",
                                         name=f"wr{n}")
                        nc.sync.dma_start(wt[:], wp_all[m])
                        w_tiles[key] = (wt[:, 0], wt[:, 1], wt[:, 2])

                    def ewin_load(jp):
                        js = bass.ds(jp * A1W, A1W)
                        h = jp % 2
                        nc.sync.dma_start(eiw[h][:], ep_i[:, :, :, js])
                        nc.sync.dma_start(er[:, :, :, js], ep_r[:, :, :, js])
                        nc.sync.dma_start(esw[h][:], ep_s[:, :, :, js])

                    js0 = bass.ds(0, A1W)
                    nc.sync.dma_start(eiw[0][:], ep_i[:, :, :, js0])
                    nc.sync.dma_start(
                        vi[:], vp_i[:].rearrange("k p s m -> p k s m"))
                    nc.sync.dma_start(er[:, :, :, js0], ep_r[:, :, :, js0])
                    nc.sync.dma_start(
                        vr[:], vp_r[:].rearrange("k p s m -> p k s m"))
                    nc.sync.dma_start(esw[0][:], ep_s[:, :, :, js0])
                    nc.sync.dma_start(
                        vs[:], vp_s[:].rearrange("k p s m -> p k s m"))
                    vd = bass.ds(0, D)

                    def a2_site(tb, ei_t, es_t):
                        tbs = bass.ts(tb, P)
                        lsl = bass.ds((tb % 4) * P, P)
                        n = uid()
                        pp = psA.tile([P, 3, D], F32, tag="pp", name=f"pp{n}")
                        dr_product(nc, pp[:, 0], pack_h0(ei_t, lsl),
                                   pack_h1(vi, vd), KC)
                        dr_product(nc, pp[:, 1], pack_h0(er, tbs),
                                   pack_h1(vr, vd), KC)
                        dr_product(nc, pp[:, 2], pack_h0(es_t, lsl),
                                   pack_h1(vs, vd), KC)
                        ppk = pkA.tile([P, 6, D], F8, tag="pk",
                                       name=f"ppk{n}")
                        site_evac(pp, D, CQ, ppk[:, 1::2], ppk[:, 0::2],
                                  evA, rcA)
                        nc.sync.dma_start(pt[tb], ppk[:])

                    def a1_site(jp, m, ei_t, es_t):
                        js = bass.ds(jp * A1W, A1W)
                        fw = bass.ds(0, A1W)
                        fp128 = bass.ds(0, P)
                        wrm, wim, wsm = w_tiles.pop((jp, m))
                        n = uid()
                        pp = psA.tile([P, 3, A1W], F32, tag="pp",
                                      name=f"pp{n}")
                        dr_product(nc, pp[:, 0], pack_h1(wim, fp128),
                                   pack_h0(ei_t, fw), KC)
                        dr_product(nc, pp[:, 1], pack_h1(wrm, fp128),
                                   pack_h0(er, js), KC)
                        dr_product(nc, pp[:, 2], pack_h1(wsm, fp128),
                                   pack_h0(es_t, fw), KC)
                        qpk = pkA.tile([P, 2, 6, SPAN], F8, tag="qpk",
                                       name=f"qpk{n}")
                        site_evac(pp, A1W, CQ,
                                  qpk[:, :, 1::2].rearrange(
                                      "p a v t -> p v a t"),
                                  qpk[:, :, 0::2].rearrange(
                                      "p a v t -> p v a t"),
                                  evA, rcA)
                        nc.sync.dma_start(q[m, 2 * jp], qpk[:, 0])
                        nc.sync.dma_start(q[m, 2 * jp + 1], qpk[:, 1])

                    pairs = [(jp, m) for jp in range(NJP) for m in range(MB)]
                    for jp in range(NJP):
                        ei_t, es_t = eiw[jp % 2], esw[jp % 2]
                        a2_site(4 * jp + 0, ei_t, es_t)
                        if jp == 0:
                            load_w((0, 0), 0)
                            load_w((0, 1), 1)
                        a2_site(4 * jp + 1, ei_t, es_t)
                        if jp == 0:
                            nc.sync.dma_start(mask_sb[:], trimask[:])
                            nc.sync.dma_start(rho_sb[:], rho2[:])
                        a2_site(4 * jp + 2, ei_t, es_t)
                        a2_site(4 * jp + 3, ei_t, es_t)
                        if jp + 1 < NJP:
                            ewin_load(jp + 1)
                        if jp in (1, 2):
                            hq = bass.ds((jp - 1) * A1W, A1W)
                            nc.sync.dma_start(nei01[:, :, :, hq],
                                              ep_ni[:, :, :, hq])
                            nc.sync.dma_start(ed01[:, :, :, hq],
                                              ep_d[:, :, :, hq])
                        for m in range(MB):
                            idx = jp * MB + m
                            if idx + 2 < len(pairs):
                                load_w(pairs[idx + 2], pairs[idx + 2][1])
                            a1_site(jp, m, ei_t, es_t)
                            if jp == 0 and m == MB - 1:
                                load_qsb(0)
                                load_qsb(1)

            # =============== Phase B: S = E^H Q (upper tri) ===============
            with tc.tile_pool(name="ptp", bufs=1) as ptpp, \
                 tc.tile_pool(name="ptsp", bufs=1) as ptsp:
                ptr = ptpp.tile([P, TB, 2, D], F8, tag="ptr")
                pti = ptpp.tile([P, TB, 2, D], F8, tag="pti")

                with tc.tile_pool(name="edp", bufs=1) as edp, \
                     tc.tile_pool(name="psB", bufs=3, space="PSUM") as psB, \
                     tc.tile_pool(name="rcB", bufs=3) as rcB, \
                     tc.tile_pool(name="evB", bufs=3) as evB, \
                     tc.tile_pool(name="spkp", bufs=3) as spkp:
                    nei23 = edp.tile([P, KC, 2, T // 2], F8, tag="nei23")
                    ed23 = edp.tile([P, KC, 2, T // 2], F8, tag="ed23")
                    nc.sync.dma_start(nei23[:], ep_ni[:, :, :, T // 2:])
                    nc.sync.dma_start(ed23[:], ep_d[:, :, :, T // 2:])
                    pts = ptsp.tile([P, TB, 2, D], F8, tag="pts")
                    nc.sync.dma_start(
                        pts[:], pt[:, :, 4:6].rearrange("t p v d -> p t v d"))

                    def b_lhs(t01, t23, ib):
                        if ib < MB:
                            return pack_h0(t01, bass.ts(ib, P))
                        return pack_h0(t23, bass.ts(ib - MB, P))
                    nc.sync.dma_start(
                        ptr[:], pt[:, :, 0:2].rearrange("t p v d -> p t v d"))
                    nc.sync.dma_start(
                        pti[:], pt[:, :, 2:4].rearrange("t p v d -> p t v d"))

                    for sp in range(NSP):
                        if sp + 2 < NSP:
                            load_qsb(sp + 2)
                        qsb = qsb_tiles.pop(sp)

                        def rhs_q(vb):
                            def f(k, kind):
                                if kind == "hh":
                                    return qsb[:, 2 * k:2 * k + 2, vb + 1, :]
                                return qsb[:, k, vb:vb + 2, :]
                            return f

                        for ib in range(2 * sp + 2):
                            ibs = bass.ts(ib, P)
                            n = uid()
                            pp = psB.tile([P, 3, SPAN], F32, tag="pp",
                                          name=f"pp{n}")
                            dr_product(nc, pp[:, 0], b_lhs(nei01, nei23, ib),
                                       rhs_q(2), KC)
                            dr_product(nc, pp[:, 1], pack_h0(er, ibs),
                                       rhs_q(0), KC)
                            dr_product(nc, pp[:, 2], b_lhs(ed01, ed23, ib),
                                       rhs_q(4), KC)
                            masks = [bass.ds(jh * P, P) for jh in range(2)
                                     if ib == 2 * sp + jh]
                            spk = spkp.tile([P, 2, 6, P], F8, tag="spk",
                                            name=f"spk{n}")
                            h_ap = spk[:, :, 0::2].rearrange(
                                "p a v j -> p v a j")
                            l_ap = spk[:, :, 1::2].rearrange(
                                "p a v j -> p v a j")
                            site_evac(pp, SPAN, CS, h_ap, l_ap,
                                      evB, rcB, masks=masks)
                            for jh in range(2):
                                jb = 2 * sp + jh
                                if ib <= jb:
                                    nc.sync.dma_start(s[ib, jb],
                                                      spk[:, jh])

                # =============== Phase C (descending jb) ===============
                with tc.tile_pool(name="sstp", bufs=2) as sstp, \
                     tc.tile_pool(name="psC", bufs=2, space="PSUM") as psC, \
                     tc.tile_pool(name="rcC", bufs=2) as rcC, \
                     tc.tile_pool(name="evC", bufs=2) as evC, \
                     tc.tile_pool(name="out4", bufs=3) as out4:
                    vd = bass.ds(0, D)
                    sst_tiles = {}

                    def load_sst(jb):
                        t = sstp.tile([P, TB, 6, P], F8, tag="sst",
                                      name=f"sst{jb}")[:, :jb + 1]
                        nc.sync.dma_start(
                            t[:], s[:jb + 1, jb].rearrange(
                                "i p v j -> p i v j"))
                        sst_tiles[jb] = t

                    load_sst(0)
                    for jb in range(TB):
                        jbs = bass.ts(jb, P)
                        nk = jb + 1
                        if jb + 1 < TB:
                            load_sst(jb + 1)
                        sst = sst_tiles.pop(jb)

                        def lhs_s(vb):
                            def f(k, kind):
                                if kind == "hh":
                                    return sst[:, 2 * k:2 * k + 2, vb, :]
                                return sst[:, k, vb:vb + 2, :]
                            return f

                        n = uid()
                        pp = psC.tile([P, 3, D], F32, tag="pp", name=f"pp{n}")
                        kl = nk - 1
                        dr_product(nc, pp[:, 0], lhs_s(2), pack_h1(pti, vd),
                                   nk, leftover=(sst[:, kl, 2, :],
                                                 pti[:, kl, 1, vd]))
                        dr_product(nc, pp[:, 1], lhs_s(0), pack_h1(ptr, vd),
                                   nk, leftover=(sst[:, kl, 0, :],
                                                 ptr[:, kl, 1, vd]))
                        dr_product(nc, pp[:, 2], lhs_s(4), pack_h1(pts, vd),
                                   nk, leftover=(sst[:, kl, 4, :],
                                                 pts[:, kl, 1, vd]))
                        ev = site_evac(pp, D, None, None, None, evC, rcC)
                        oo = out4.tile([P, 2, D], F32, tag="oo",
                                       name=f"oo{jb}")
                        nc.scalar.activation(oo[:], ev[:, 0:2], COPY,
                                             scale=rho_sb[:, jb:jb + 1])
                        nc.sync.dma_start(outT_re[jbs, :], oo[:, 0])
                        nc.sync.dma_start(outT_im[jbs, :], oo[:, 1])
            bd01_cm.__exit__(None, None, None)
            qsbp_cm.__exit__(None, None, None)

    nc.compile()
    return nc


_NC_CACHE = None


def _get_module():
    global _NC_CACHE
    if _NC_CACHE is None:
        _NC_CACHE = build_module()
    return _NC_CACHE


def _split(x):
    h = x.astype(E4NP)
    l = (x - h.astype(np.float32)).astype(E4NP)
    return h, l


def _pack(x, hfirst):
    """x [D2, F] f32 -> fp8 pack: [P, KC, 2, F] (E, h-first) or
    [KC, P, 2, F] (weights, l-first)."""
    h, l = _split(x)
    F = x.shape[1]
    if hfirst:
        out = np.empty((P, KC, 2, F), E4NP)
        out[:, :, 0] = h.reshape(KC, P, F).transpose(1, 0, 2)
        out[:, :, 1] = l.reshape(KC, P, F).transpose(1, 0, 2)
    else:
        out = np.empty((KC, P, 2, F), E4NP)
        out[:, :, 1] = h.reshape(KC, P, F)
        out[:, :, 0] = l.reshape(KC, P, F)
    return out


def _pack_w(w):
    """w [D2, D2] (c, m) f32 -> [MB, P, KC, 2(l,h), P] fp8 pack."""
    h, l = _split(w)
    out = np.empty((MB, P, KC, 2, P), E4NP)
    out[:, :, :, 1] = h.reshape(KC, P, MB, P).transpose(2, 1, 0, 3)
    out[:, :, :, 0] = l.reshape(KC, P, MB, P).transpose(2, 1, 0, 3)
    return out


def prep_shared(WKQ_re, WKQ_im, WPV_re, WPV_im):
    wr = np.ascontiguousarray(WKQ_re.T) * 256.0
    wi = np.ascontiguousarray(WKQ_im.T) * 256.0
    vr = np.ascontiguousarray(WPV_re.T) * 256.0
    vi = np.ascontiguousarray(WPV_im.T) * 256.0
    wall = np.stack([_pack_w(wr), _pack_w(wi), _pack_w(wr + wi)], axis=2)
    shared = {
        "wp_all": np.ascontiguousarray(wall),
        "vp_r": _pack(vr, False), "vp_i": _pack(vi, False),
        "vp_s": _pack(vr + vi, False),
        "trimask": np.triu(np.ones((P, P), np.float32)),
    }
    j = np.arange(T, dtype=np.float32)
    rho = 2.0 / np.maximum(j, 1.0)
    shared["rho2"] = np.ascontiguousarray(rho.reshape(TB, P).T)
    return shared


def kernel(E_re, E_im, WKQ_re, WKQ_im, WPV_re, WPV_im):
    E_re = np.asarray(E_re, dtype=np.float32)
    E_im = np.asarray(E_im, dtype=np.float32)
    shared = prep_shared(np.asarray(WKQ_re, np.float32),
                         np.asarray(WKQ_im, np.float32),
                         np.asarray(WPV_re, np.float32),
                         np.asarray(WPV_im, np.float32))
    in_maps = []
    for b in range(B):
        er = E_re[b] * 4.0
        ei = E_im[b] * 4.0
        m = dict(shared)
        m["ep_r"] = _pack(er, True)
        m["ep_i"] = _pack(ei, True)
        m["ep_ni"] = _pack(-ei, True)
        m["ep_s"] = _pack(er + ei, True)
        m["ep_d"] = _pack(er - ei, True)
        in_maps.append(m)

    nc = _get_module()
    res = run_bass_kernel_spmd(nc, in_maps, core_ids=list(range(B)))

    out = np.empty((B, D, T - 2), dtype=np.complex64)
    for b in range(B):
        r = res.results[b]["outT_re"]  # [T, D]
        i = res.results[b]["outT_im"]
        full = (r + 1j * i.astype(np.complex64)).T  # [D, T]
        out[b] = full[:, 1:T - 1]
    return out


# revision 31
# speedup vs baseline: 1.0286x; 1.0286x over previous
"""Trainium2 Bass kernel for nn_AutoregressiveLSA — fp8 DoubleRow version.

Math (complex, per batch b, one NeuronCore per batch element):
    Q  = WKQ @ E                       [2d, T]
    S  = E^H @ Q, keep i <= j          [T, T]
    outT[j] = sum_{i<=j} S[i,j] PT[i] * 2/max(j,1),  PT = (WPV @ E)^T

All matmuls run as fp8e4 (e4m3) in DoubleRow perf mode: one PE
instruction contracts TWO 128-chunks at 0.5 cycles/output-column (4x
the fp32r MAC rate).  Precision comes from a hi/lo split of every
operand (x ~ x_h + x_l, both e4m3; x_l*y_l dropped): per 128-chunk each
real product needs 3 fp8 pairings = 1.5 DR instructions, so a complex
Karatsuba product costs 2.25 free-columns/chunk vs 3.0 for fp32r.
Measured end-to-end rel err ~3e-3 (gate 2e-2).

Scale chain (powers of 2, folded into casts / final rho):
    E*4, WKQ^T*256, WPV^T*256 quantized on host.
    A1 psum = 1024*Q,  split scale 2^-7  -> Q'' = 8Q
    A2 psum = 1024*PT, split scale 2^-7  -> PT'' = 8PT
    B  psum = 32*S,    split scale 2^-9  -> S'' = S/16
    C  psum = S*PT/2,  rho2 = 2/max(j,1) applied via Act scale.

Engine constraints honored (probed on real TRN2): vector ops may read
at most ONE psum operand; Pool (gpsimd) runs SBUF-only tensor_tensor
(no psum, no scalar_tensor_tensor); Act does scaled copies (fp8 out ok).
Evacuation is fused into wide ops: psum banks ordered (M2, M1, M3) so
one 3W psum->sbuf copy + one dual-sub [re,tt] + pool im/sum + ONE 3W
Act h-cast + ONE 3W DVE stt l-split handle a whole complex site.
Phase B uses a host-negated E_im pack (nei) so its conjugated
recombination has the same (M1-M2', M3-M1-M2') form as the others.
"""

import numpy as np
import ml_dtypes

import concourse.bass as bass
import concourse.mybir as mybir
import concourse.tile as tile
from concourse import bacc
from concourse.bass_utils import run_bass_kernel_spmd
from concourse.alu_op_type import AluOpType

F32 = mybir.dt.float32
F8 = mybir.dt.float8e4
E4NP = ml_dtypes.float8_e4m3
DR = mybir.MatmulPerfMode.DoubleRow
COPY = mybir.ActivationFunctionType.Copy

B = 8
D2 = 1024
T = 2048
D = 512
P = 128
KC = D2 // P
MB = D2 // P
TB = T // P
A1W = 512
NJP = T // A1W
SPAN = 256
NSP = T // SPAN

CQ = float(2.0 ** -7)
CS = float(2.0 ** -9)


def pack_h0(t, fsl):
    """Slicer for h-first packs [P, K, 2(h,l), F] (E/S side)."""
    def f(k, kind):
        if kind == "hh":
            return t[:, 2 * k:2 * k + 2, 0, fsl]
        return t[:, k, :, fsl]
    return f


def pack_h1(t, fsl):
    """Slicer for l-first packs [P, K, 2(l,h), F] (W/Q/PT side)."""
    def f(k, kind):
        if kind == "hh":
            return t[:, 2 * k:2 * k + 2, 1, fsl]
        return t[:, k, :, fsl]
    return f


def dr_product(nc, bank, lhs, rhs, nk, leftover=None):
    nhh = nk // 2
    odd = nk % 2
    tot = nhh + nk + (1 if odd else 0)
    i = 0
    for kp in range(nhh):
        nc.tensor.matmul(bank, lhs(kp, "hh"), rhs(kp, "hh"),
                         start=(i == 0), stop=(i == tot - 1), perf_mode=DR)
        i += 1
    for k in range(nk):
        nc.tensor.matmul(bank, lhs(k, "x"), rhs(k, "x"),
                         start=(i == 0), stop=(i == tot - 1), perf_mode=DR)
        i += 1
    if odd:
        la, ra = leftover
        nc.tensor.matmul(bank, la, ra, start=(i == 0), stop=(i == tot - 1))


def build_module():
    nc = bacc.Bacc(target_bir_lowering=False, trn_type="TRN2")

    ep_r = nc.dram_tensor("ep_r", [P, KC, 2, T], F8, kind="ExternalInput")
    ep_i = nc.dram_tensor("ep_i", [P, KC, 2, T], F8, kind="ExternalInput")
    ep_ni = nc.dram_tensor("ep_ni", [P, KC, 2, T], F8, kind="ExternalInput")
    ep_s = nc.dram_tensor("ep_s", [P, KC, 2, T], F8, kind="ExternalInput")
    ep_d = nc.dram_tensor("ep_d", [P, KC, 2, T], F8, kind="ExternalInput")
    wp_r = nc.dram_tensor("wp_r", [MB, P, KC, 2, P], F8, kind="ExternalInput")
    wp_i = nc.dram_tensor("wp_i", [MB, P, KC, 2, P], F8, kind="ExternalInput")
    wp_s = nc.dram_tensor("wp_s", [MB, P, KC, 2, P], F8, kind="ExternalInput")
    vp_r = nc.dram_tensor("vp_r", [KC, P, 2, D], F8, kind="ExternalInput")
    vp_i = nc.dram_tensor("vp_i", [KC, P, 2, D], F8, kind="ExternalInput")
    vp_s = nc.dram_tensor("vp_s", [KC, P, 2, D], F8, kind="ExternalInput")
    trimask = nc.dram_tensor("trimask", [P, P], F32, kind="ExternalInput")
    rho2 = nc.dram_tensor("rho2", [P, TB], F32, kind="ExternalInput")
    outT_re = nc.dram_tensor("outT_re", [T, D], F32, kind="ExternalOutput")
    outT_im = nc.dram_tensor("outT_im", [T, D], F32, kind="ExternalOutput")

    _n = [0]

    def uid():
        _n[0] += 1
        return _n[0]

    with tile.TileContext(nc) as tc:
        with tc.tile_pool(name="dram", bufs=1, space="DRAM") as dram, \
             tc.tile_pool(name="erp", bufs=1) as erp, \
             tc.tile_pool(name="cst", bufs=1) as cst:
            q = dram.tile([MB, NSP, P, 6, SPAN], F8, tag="q")
            pt = dram.tile([TB, P, 6, D], F8, tag="pt")
            s = dram.tile([TB, TB, P, 6, P], F8, tag="s")

            er = erp.tile([P, KC, 2, T], F8, tag="er")
            mask_sb = cst.tile([P, P], F32, tag="mask")
            rho_sb = cst.tile([P, TB], F32, tag="rho")

            def site_evac(pp, width, c, pk_h_ap, pk_l_ap, ev_pool, rc_pool,
                          masks=None):
                """Evacuate one complex site.

                pp: psum tile [P, 3, width] with banks (M2, M1, M3).
                pk_h_ap/pk_l_ap: output APs for h/l fp8 splits of
                (re, im, sum), or None to skip splits (phase C).
                Returns ev tile [P, 4, width] = (re, im, sum, tt).
                """
                n = uid()
                rc = rc_pool.tile([P, 3, width], F32, tag="rc", name=f"rc{n}")
                ev = ev_pool.tile([P, 4, width], F32, tag="ev", name=f"ev{n}")
                nc.scalar.activation(rc[:], pp[:], COPY)
                nc.vector.tensor_sub(ev[:, 0::3], rc[:, 1:3], rc[:, 0:2])
                nc.gpsimd.tensor_sub(ev[:, 1], ev[:, 3], rc[:, 0])
                if masks is not None:
                    for dsl in masks:
                        nc.vector.tensor_mul(ev[:, 0, dsl], ev[:, 0, dsl],
                                             mask_sb[:])
                        nc.vector.tensor_mul(ev[:, 1, dsl], ev[:, 1, dsl],
                                             mask_sb[:])
                if pk_h_ap is None:
                    return ev
                nc.gpsimd.tensor_add(ev[:, 2], ev[:, 0], ev[:, 1])
                nc.scalar.activation(pk_h_ap, ev[:, 0:3], COPY, scale=c)
                nc.vector.scalar_tensor_tensor(
                    out=pk_l_ap, in0=ev[:, 0:3], scalar=c, in1=pk_h_ap,
                    op0=AluOpType.mult, op1=AluOpType.subtract)
                return ev

            # =============== Phases A1 + A2 (merged psum scope) ===========
            qsbp_cm = tc.tile_pool(name="qsbp", bufs=2)
            qsbp = qsbp_cm.__enter__()
            bd01_cm = tc.tile_pool(name="bd01", bufs=1)
            bd01 = bd01_cm.__enter__()
            nei01 = bd01.tile([P, KC, 2, T // 2], F8, tag="nei01")
            ed01 = bd01.tile([P, KC, 2, T // 2], F8, tag="ed01")
            qsb_tiles = {}

            def load_qsb(sp):
                t = qsbp.tile([P, MB, 6, SPAN], F8, tag="qsb",
                              name=f"qsb{sp}")
                nc.sync.dma_start(
                    t[:], q[:, sp].rearrange("m p v t -> p m v t"))
                qsb_tiles[sp] = t

            with tc.tile_pool(name="eip", bufs=1) as eip, \
                 tc.tile_pool(name="esp", bufs=1) as esp:
                # ei/es are rolling 2-panel windows (A2+A1 consume jp-wise)
                eiw = [eip.tile([P, KC, 2, A1W], F8, tag=f"eiw{h}",
                                name=f"eiw{h}") for h in range(2)]
                esw = [esp.tile([P, KC, 2, A1W], F8, tag=f"esw{h}",
                                name=f"esw{h}") for h in range(2)]

                with tc.tile_pool(name="psA", bufs=2, space="PSUM") as psA, \
                     tc.tile_pool(name="rcA", bufs=2) as rcA, \
                     tc.tile_pool(name="evA", bufs=2) as evA, \
                     tc.tile_pool(name="pkA", bufs=3) as pkA, \
                     tc.tile_pool(name="wroll", bufs=2) as wrollp, \
                     tc.tile_pool(name="vres", bufs=1) as vres:
                    vr = vres.tile([P, KC, 2, D], F8, tag="vr")
                    vi = vres.tile([P, KC, 2, D], F8, tag="vi")
                    vs = vres.tile([P, KC, 2, D], F8, tag="vs")

                    w_tiles = {}

                    def load_w(key, m):
                        n = uid()
                        wrm = wrollp.tile([P, KC, 2, P], F8, tag="wr",
                                          name=f"wr{n}")
                        wim = wrollp.tile([P, KC, 2, P], F8, tag="wi",
                                          name=f"wi{n}")
                        wsm = wrollp.tile([P, KC, 2, P], F8, tag="ws",
                                          name=f"ws{n}")
                        nc.sync.dma_start(wrm[:], wp_r[m])
                        nc.sync.dma_start(wim[:], wp_i[m])
                        nc.sync.dma_start(wsm[:], wp_s[m])
                        w_tiles[key] = (wrm, wim, wsm)

                    def ewin_load(jp):
                        js = bass.ds(jp * A1W, A1W)
                        h = jp % 2
                        nc.sync.dma_start(eiw[h][:], ep_i[:, :, :, js])
                        nc.sync.dma_start(er[:, :, :, js], ep_r[:, :, :, js])
                        nc.sync.dma_start(esw[h][:], ep_s[:, :, :, js])

                    js0 = bass.ds(0, A1W)
                    nc.sync.dma_start(eiw[0][:], ep_i[:, :, :, js0])
                    nc.sync.dma_start(
                        vi[:], vp_i[:].rearrange("k p s m -> p k s m"))
                    nc.sync.dma_start(er[:, :, :, js0], ep_r[:, :, :, js0])
                    nc.sync.dma_start(
                        vr[:], vp_r[:].rearrange("k p s m -> p k s m"))
                    nc.sync.dma_start(esw[0][:], ep_s[:, :, :, js0])
                    nc.sync.dma_start(
                        vs[:], vp_s[:].rearrange("k p s m -> p k s m"))
                    vd = bass.ds(0, D)

                    def a2_site(tb, ei_t, es_t):
                        tbs = bass.ts(tb, P)
                        lsl = bass.ds((tb % 4) * P, P)
                        n = uid()
                        pp = psA.tile([P, 3, D], F32, tag="pp", name=f"pp{n}")
                        dr_product(nc, pp[:, 0], pack_h0(ei_t, lsl),
                                   pack_h1(vi, vd), KC)
                        dr_product(nc, pp[:, 1], pack_h0(er, tbs),
                                   pack_h1(vr, vd), KC)
                        dr_product(nc, pp[:, 2], pack_h0(es_t, lsl),
                                   pack_h1(vs, vd), KC)
                        ppk = pkA.tile([P, 6, D], F8, tag="pk",
                                       name=f"ppk{n}")
                        site_evac(pp, D, CQ, ppk[:, 1::2], ppk[:, 0::2],
                                  evA, rcA)
                        nc.sync.dma_start(pt[tb], ppk[:])

                    def a1_site(jp, m, ei_t, es_t):
                        js = bass.ds(jp * A1W, A1W)
                        fw = bass.ds(0, A1W)
                        fp128 = bass.ds(0, P)
                        wrm, wim, wsm = w_tiles.pop((jp, m))
                        n = uid()
                        pp = psA.tile([P, 3, A1W], F32, tag="pp",
                                      name=f"pp{n}")
                        dr_product(nc, pp[:, 0], pack_h1(wim, fp128),
                                   pack_h0(ei_t, fw), KC)
                        dr_product(nc, pp[:, 1], pack_h1(wrm, fp128),
                                   pack_h0(er, js), KC)
                        dr_product(nc, pp[:, 2], pack_h1(wsm, fp128),
                                   pack_h0(es_t, fw), KC)
                        qpk = pkA.tile([P, 2, 6, SPAN], F8, tag="qpk",
                                       name=f"qpk{n}")
                        site_evac(pp, A1W, CQ,
                                  qpk[:, :, 1::2].rearrange(
                                      "p a v t -> p v a t"),
                                  qpk[:, :, 0::2].rearrange(
                                      "p a v t -> p v a t"),
                                  evA, rcA)
                        nc.sync.dma_start(q[m, 2 * jp], qpk[:, 0])
                        nc.sync.dma_start(q[m, 2 * jp + 1], qpk[:, 1])

                    pairs = [(jp, m) for jp in range(NJP) for m in range(MB)]
                    for jp in range(NJP):
                        ei_t, es_t = eiw[jp % 2], esw[jp % 2]
                        a2_site(4 * jp + 0, ei_t, es_t)
                        if jp == 0:
                            load_w((0, 0), 0)
                            load_w((0, 1), 1)
                        a2_site(4 * jp + 1, ei_t, es_t)
                        if jp == 0:
                            nc.sync.dma_start(mask_sb[:], trimask[:])
                            nc.sync.dma_start(rho_sb[:], rho2[:])
                        a2_site(4 * jp + 2, ei_t, es_t)
                        a2_site(4 * jp + 3, ei_t, es_t)
                        if jp + 1 < NJP:
                            ewin_load(jp + 1)
                        if jp in (1, 2):
                            hq = bass.ds((jp - 1) * A1W, A1W)
                            nc.sync.dma_start(nei01[:, :, :, hq],
                                              ep_ni[:, :, :, hq])
                            nc.sync.dma_start(ed01[:, :, :, hq],
                                              ep_d[:, :, :, hq])
                        for m in range(MB):
                            idx = jp * MB + m
                            if idx + 2 < len(pairs):
                                load_w(pairs[idx + 2], pairs[idx + 2][1])
                            a1_site(jp, m, ei_t, es_t)
                            if jp == 0 and m == MB - 1:
                                load_qsb(0)
                                load_qsb(1)

            # =============== Phase B: S = E^H Q (upper tri) ===============
            with tc.tile_pool(name="ptp", bufs=1) as ptpp:
                ptr = ptpp.tile([P, TB, 2, D], F8, tag="ptr")
                pti = ptpp.tile([P, TB, 2, D], F8, tag="pti")

                with tc.tile_pool(name="edp", bufs=1) as edp, \
                     tc.tile_pool(name="psB", bufs=3, space="PSUM") as psB, \
                     tc.tile_pool(name="rcB", bufs=3) as rcB, \
                     tc.tile_pool(name="evB", bufs=3) as evB, \
                     tc.tile_pool(name="spkp", bufs=3) as spkp:
                    nei23 = edp.tile([P, KC, 2, T // 2], F8, tag="nei23")
                    ed23 = edp.tile([P, KC, 2, T // 2], F8, tag="ed23")

                    def b_lhs(t01, t23, ib):
                        if ib < MB:
                            return pack_h0(t01, bass.ts(ib, P))
                        return pack_h0(t23, bass.ts(ib - MB, P))
                    nc.sync.dma_start(
                        ptr[:], pt[:, :, 0:2].rearrange("t p v d -> p t v d"))
                    nc.sync.dma_start(
                        pti[:], pt[:, :, 2:4].rearrange("t p v d -> p t v d"))

                    for sp in range(NSP):
                        if sp + 2 < NSP:
                            load_qsb(sp + 2)
                        if sp < 2:
                            lq = bass.ds(sp * A1W, A1W)
                            gq = bass.ds(T // 2 + sp * A1W, A1W)
                            nc.sync.dma_start(nei23[:, :, :, lq],
                                              ep_ni[:, :, :, gq])
                            nc.sync.dma_start(ed23[:, :, :, lq],
                                              ep_d[:, :, :, gq])
                        qsb = qsb_tiles.pop(sp)

                        def rhs_q(vb):
                            def f(k, kind):
                                if kind == "hh":
                                    return qsb[:, 2 * k:2 * k + 2, vb + 1, :]
                                return qsb[:, k, vb:vb + 2, :]
                            return f

                        for ib in range(2 * sp + 2):
                            ibs = bass.ts(ib, P)
                            n = uid()
                            pp = psB.tile([P, 3, SPAN], F32, tag="pp",
                                          name=f"pp{n}")
                            dr_product(nc, pp[:, 0], b_lhs(nei01, nei23, ib),
                                       rhs_q(2), KC)
                            dr_product(nc, pp[:, 1], pack_h0(er, ibs),
                                       rhs_q(0), KC)
                            dr_product(nc, pp[:, 2], b_lhs(ed01, ed23, ib),
                                       rhs_q(4), KC)
                            masks = [bass.ds(jh * P, P) for jh in range(2)
                                     if ib == 2 * sp + jh]
                            spk = spkp.tile([P, 2, 6, P], F8, tag="spk",
                                            name=f"spk{n}")
                            h_ap = spk[:, :, 0::2].rearrange(
                                "p a v j -> p v a j")
                            l_ap = spk[:, :, 1::2].rearrange(
                                "p a v j -> p v a j")
                            site_evac(pp, SPAN, CS, h_ap, l_ap,
                                      evB, rcB, masks=masks)
                            for jh in range(2):
                                jb = 2 * sp + jh
                                if ib <= jb:
                                    nc.sync.dma_start(s[ib, jb],
                                                      spk[:, jh])

                # =============== Phase C (descending jb) ===============
                with tc.tile_pool(name="ptsp", bufs=1) as ptsp, \
                     tc.tile_pool(name="sstp", bufs=2) as sstp, \
                     tc.tile_pool(name="psC", bufs=2, space="PSUM") as psC, \
                     tc.tile_pool(name="rcC", bufs=2) as rcC, \
                     tc.tile_pool(name="evC", bufs=2) as evC, \
                     tc.tile_pool(name="out4", bufs=3) as out4:
                    pts = ptsp.tile([P, TB, 2, D], F8, tag="pts")
                    nc.sync.dma_start(
                        pts[:], pt[:, :, 4:6].rearrange("t p v d -> p t v d"))
                    vd = bass.ds(0, D)
                    sst_tiles = {}

                    def load_sst(jb):
                        t = sstp.tile([P, TB, 6, P], F8, tag="sst",
                                      name=f"sst{jb}")[:, :jb + 1]
                        nc.sync.dma_start(
                            t[:], s[:jb + 1, jb].rearrange(
                                "i p v j -> p i v j"))
                        sst_tiles[jb] = t

                    load_sst(0)
                    for jb in range(TB):
                        jbs = bass.ts(jb, P)
                        nk = jb + 1
                        if jb + 1 < TB:
                            load_sst(jb + 1)
                        sst = sst_tiles.pop(jb)

                        def lhs_s(vb):
                            def f(k, kind):
                                if kind == "hh":
                                    return sst[:, 2 * k:2 * k + 2, vb, :]
                                return sst[:, k, vb:vb + 2, :]
                            return f

                        n = uid()
                        pp = psC.tile([P, 3, D], F32, tag="pp", name=f"pp{n}")
                        kl = nk - 1
                        dr_product(nc, pp[:, 0], lhs_s(2), pack_h1(pti, vd),
                                   nk, leftover=(sst[:, kl, 2, :],
                                                 pti[:, kl, 1, vd]))
                        dr_product(nc, pp[:, 1], lhs_s(0), pack_h1(ptr, vd),
                                   nk, leftover=(sst[:, kl, 0, :],
                                                 ptr[:, kl, 1, vd]))
                        dr_product(nc, pp[:, 2], lhs_s(4), pack_h1(pts, vd),
                                   nk, leftover=(sst[:, kl, 4, :],
                                                 pts[:, kl, 1, vd]))
                        ev = site_evac(pp, D, None, None, None, evC, rcC)
                        oo = out4.tile([P, 2, D], F32, tag="oo",
                                       name=f"oo{jb}")
                        nc.scalar.activation(oo[:], ev[:, 0:2], COPY,
                                             scale=rho_sb[:, jb:jb + 1])
                        nc.sync.dma_start(outT_re[jbs, :], oo[:, 0])
                        nc.sync.dma_start(outT_im[jbs, :], oo[:, 1])
            bd01_cm.__exit__(None, None, None)
            qsbp_cm.__exit__(None, None, None)

    nc.compile()
    return nc


_NC_CACHE = None


def _get_module():
    global _NC_CACHE
    if _NC_CACHE is None:
        _NC_CACHE = build_module()
    return _NC_CACHE


def _split(x):
    h = x.astype(E4NP)
    l = (x - h.astype(np.float32)).astype(E4NP)
    return h, l


def _pack(x, hfirst):
    """x [D2, F] f32 -> fp8 pack: [P, KC, 2, F] (E, h-first) or
    [KC, P, 2, F] (weights, l-first)."""
    h, l = _split(x)
    F = x.shape[1]
    if hfirst:
        out = np.empty((P, KC, 2, F), E4NP)
        out[:, :, 0] = h.reshape(KC, P, F).transpose(1, 0, 2)
        out[:, :, 1] = l.reshape(KC, P, F).transpose(1, 0, 2)
    else:
        out = np.empty((KC, P, 2, F), E4NP)
        out[:, :, 1] = h.reshape(KC, P, F)
        out[:, :, 0] = l.reshape(KC, P, F)
    return out


def _pack_w(w):
    """w [D2, D2] (c, m) f32 -> [MB, P, KC, 2(l,h), P] fp8 pack."""
    h, l = _split(w)
    out = np.empty((MB, P, KC, 2, P), E4NP)
    out[:, :, :, 1] = h.reshape(KC, P, MB, P).transpose(2, 1, 0, 3)
    out[:, :, :, 0] = l.reshape(KC, P, MB, P).transpose(2, 1, 0, 3)
    return out


def prep_shared(WKQ_re, WKQ_im, WPV_re, WPV_im):
    wr = np.ascontiguousarray(WKQ_re.T) * 256.0
    wi = np.ascontiguousarray(WKQ_im.T) * 256.0
    vr = np.ascontiguousarray(WPV_re.T) * 256.0
    vi = np.ascontiguousarray(WPV_im.T) * 256.0
    shared = {
        "wp_r": _pack_w(wr), "wp_i": _pack_w(wi),
        "wp_s": _pack_w(wr + wi),
        "vp_r": _pack(vr, False), "vp_i": _pack(vi, False),
        "vp_s": _pack(vr + vi, False),
        "trimask": np.triu(np.ones((P, P), np.float32)),
    }
    j = np.arange(T, dtype=np.float32)
    rho = 2.0 / np.maximum(j, 1.0)
    shared["rho2"] = np.ascontiguousarray(rho.reshape(TB, P).T)
    return shared


def kernel(E_re, E_im, WKQ_re, WKQ_im, WPV_re, WPV_im):
    E_re = np.asarray(E_re, dtype=np.float32)
    E_im = np.asarray(E_im, dtype=np.float32)
    shared = prep_shared(np.asarray(WKQ_re, np.float32),
                         np.asarray(WKQ_im, np.float32),
                         np.asarray(WPV_re, np.float32),
                         np.asarray(WPV_im, np.float32))
    in_maps = []
    for b in range(B):
        er = E_re[b] * 4.0
        ei = E_im[b] * 4.0
        m = dict(shared)
        m["ep_r"] = _pack(er, True)
        m["ep_i"] = _pack(ei, True)
        m["ep_ni"] = _pack(-ei, True)
        m["ep_s"] = _pack(er + ei, True)
        m["ep_d"] = _pack(er - ei, True)
        in_maps.append(m)

    nc = _get_module()
    res = run_bass_kernel_spmd(nc, in_maps, core_ids=list(range(B)))

    out = np.empty((B, D, T - 2), dtype=np.complex64)
    for b in range(B):
        r = res.results[b]["outT_re"]  # [T, D]
        i = res.results[b]["outT_im"]
        full = (r + 1j * i.astype(np.complex64)).T  # [D, T]
        out[b] = full[:, 1:T - 1]
    return out


# revision 39
# speedup vs baseline: 1.0400x; 1.0111x over previous
"""Trainium2 Bass kernel for nn_AutoregressiveLSA — fp8 DoubleRow version.

Math (complex, per batch b, one NeuronCore per batch element):
    Q  = WKQ @ E                       [2d, T]
    S  = E^H @ Q, keep i <= j          [T, T]
    outT[j] = sum_{i<=j} S[i,j] PT[i] * 2/max(j,1),  PT = (WPV @ E)^T

All matmuls run as fp8e4 (e4m3) in DoubleRow perf mode: one PE
instruction contracts TWO 128-chunks at 0.5 cycles/output-column (4x
the fp32r MAC rate).  Precision comes from a hi/lo split of every
operand (x ~ x_h + x_l, both e4m3; x_l*y_l dropped): per 128-chunk each
real product needs 3 fp8 pairings = 1.5 DR instructions, so a complex
Karatsuba product costs 2.25 free-columns/chunk vs 3.0 for fp32r.
Measured end-to-end rel err ~3e-3 (gate 2e-2).

Scale chain (powers of 2, folded into casts / final rho):
    E*4, WKQ^T*256, WPV^T*256 quantized on host.
    A1 psum = 1024*Q,  split scale 2^-7  -> Q'' = 8Q
    A2 psum = 1024*PT, split scale 2^-7  -> PT'' = 8PT
    B  psum = 32*S,    split scale 2^-9  -> S'' = S/16
    C  psum = S*PT/2,  rho2 = 2/max(j,1) applied via Act scale.

Engine constraints honored (probed on real TRN2): vector ops may read
at most ONE psum operand; Pool (gpsimd) runs SBUF-only tensor_tensor
(no psum, no scalar_tensor_tensor); Act does scaled copies (fp8 out ok).
Evacuation is fused into wide ops: psum banks ordered (M2, M1, M3) so
one 3W psum->sbuf copy + one dual-sub [re,tt] + pool im/sum + ONE 3W
Act h-cast + ONE 3W DVE stt l-split handle a whole complex site.
Phase B uses a host-negated E_im pack (nei) so its conjugated
recombination has the same (M1-M2', M3-M1-M2') form as the others.
"""

import numpy as np
import ml_dtypes

import concourse.bass as bass
import concourse.mybir as mybir
import concourse.tile as tile
from concourse import bacc
from concourse.bass_utils import run_bass_kernel_spmd
from concourse.alu_op_type import AluOpType

F32 = mybir.dt.float32
F8 = mybir.dt.float8e4
E4NP = ml_dtypes.float8_e4m3
DR = mybir.MatmulPerfMode.DoubleRow
COPY = mybir.ActivationFunctionType.Copy

B = 8
D2 = 1024
T = 2048
D = 512
P = 128
KC = D2 // P
MB = D2 // P
TB = T // P
A1W = 512
NJP = T // A1W
SPAN = 256
NSP = T // SPAN

CQ = float(2.0 ** -7)
CS = float(2.0 ** -9)


def pack_h0(t, fsl):
    """Slicer for h-first packs [P, K, 2(h,l), F] (E/S side)."""
    def f(k, kind):
        if kind == "hh":
            return t[:, 2 * k:2 * k + 2, 0, fsl]
        return t[:, k, :, fsl]
    return f


def pack_h1(t, fsl):
    """Slicer for l-first packs [P, K, 2(l,h), F] (W/Q/PT side)."""
    def f(k, kind):
        if kind == "hh":
            return t[:, 2 * k:2 * k + 2, 1, fsl]
        return t[:, k, :, fsl]
    return f


def dr_product(nc, bank, lhs, rhs, nk, leftover=None):
    nhh = nk // 2
    odd = nk % 2
    tot = nhh + nk + (1 if odd else 0)
    i = 0
    for kp in range(nhh):
        nc.tensor.matmul(bank, lhs(kp, "hh"), rhs(kp, "hh"),
                         start=(i == 0), stop=(i == tot - 1), perf_mode=DR)
        i += 1
    for k in range(nk):
        nc.tensor.matmul(bank, lhs(k, "x"), rhs(k, "x"),
                         start=(i == 0), stop=(i == tot - 1), perf_mode=DR)
        i += 1
    if odd:
        la, ra = leftover
        nc.tensor.matmul(bank, la, ra, start=(i == 0), stop=(i == tot - 1))


def build_module():
    nc = bacc.Bacc(target_bir_lowering=False, trn_type="TRN2")

    ep_r = nc.dram_tensor("ep_r", [P, KC, 2, T], F8, kind="ExternalInput")
    ep_i = nc.dram_tensor("ep_i", [P, KC, 2, T], F8, kind="ExternalInput")
    ep_ni = nc.dram_tensor("ep_ni", [P, KC, 2, T], F8, kind="ExternalInput")
    ep_s = nc.dram_tensor("ep_s", [P, KC, 2, T], F8, kind="ExternalInput")
    ep_d = nc.dram_tensor("ep_d", [P, KC, 2, T], F8, kind="ExternalInput")
    wp_r = nc.dram_tensor("wp_r", [MB, P, KC, 2, P], F8, kind="ExternalInput")
    wp_i = nc.dram_tensor("wp_i", [MB, P, KC, 2, P], F8, kind="ExternalInput")
    wp_s = nc.dram_tensor("wp_s", [MB, P, KC, 2, P], F8, kind="ExternalInput")
    vp_r = nc.dram_tensor("vp_r", [KC, P, 2, D], F8, kind="ExternalInput")
    vp_i = nc.dram_tensor("vp_i", [KC, P, 2, D], F8, kind="ExternalInput")
    vp_s = nc.dram_tensor("vp_s", [KC, P, 2, D], F8, kind="ExternalInput")
    trimask = nc.dram_tensor("trimask", [P, P], F32, kind="ExternalInput")
    rho2 = nc.dram_tensor("rho2", [P, TB], F32, kind="ExternalInput")
    outT_re = nc.dram_tensor("outT_re", [T, D], F32, kind="ExternalOutput")
    outT_im = nc.dram_tensor("outT_im", [T, D], F32, kind="ExternalOutput")

    _n = [0]

    def uid():
        _n[0] += 1
        return _n[0]

    with tile.TileContext(nc) as tc:
        with tc.tile_pool(name="dram", bufs=1, space="DRAM") as dram, \
             tc.tile_pool(name="erp", bufs=1) as erp, \
             tc.tile_pool(name="cst", bufs=1) as cst:
            q = dram.tile([MB, NSP, P, 6, SPAN], F8, tag="q")
            pt = dram.tile([TB, P, 6, D], F8, tag="pt")
            s = dram.tile([TB, TB, P, 6, P], F8, tag="s")

            er = erp.tile([P, KC, 2, T], F8, tag="er")
            mask_sb = cst.tile([P, P], F32, tag="mask")
            rho_sb = cst.tile([P, TB], F32, tag="rho")

            def site_evac(pp, width, c, pk_h_ap, pk_l_ap, ev_pool, rc_pool,
                          masks=None):
                """Evacuate one complex site.

                pp: psum tile [P, 3, width] with banks (M2, M1, M3).
                pk_h_ap/pk_l_ap: output APs for h/l fp8 splits of
                (re, im, sum), or None to skip splits (phase C).
                Returns ev tile [P, 4, width] = (re, im, sum, tt).
                """
                n = uid()
                rc = rc_pool.tile([P, 3, width], F32, tag="rc", name=f"rc{n}")
                ev = ev_pool.tile([P, 4, width], F32, tag="ev", name=f"ev{n}")
                nc.scalar.activation(rc[:], pp[:], COPY)
                nc.vector.tensor_sub(ev[:, 0::3], rc[:, 1:3], rc[:, 0:2])
                nc.gpsimd.tensor_sub(ev[:, 1], ev[:, 3], rc[:, 0])
                if masks is not None:
                    for dsl in masks:
                        nc.vector.tensor_mul(ev[:, 0, dsl], ev[:, 0, dsl],
                                             mask_sb[:])
                        nc.vector.tensor_mul(ev[:, 1, dsl], ev[:, 1, dsl],
                                             mask_sb[:])
                if pk_h_ap is None:
                    return ev
                nc.gpsimd.tensor_add(ev[:, 2], ev[:, 0], ev[:, 1])
                pieces = pk_h_ap if isinstance(pk_h_ap, list) \
                    else [(pk_h_ap, pk_l_ap, slice(None))]
                for h_ap, l_ap, csl in pieces:
                    nc.scalar.activation(h_ap, ev[:, 0:3, csl], COPY, scale=c)
                    nc.vector.scalar_tensor_tensor(
                        out=l_ap, in0=ev[:, 0:3, csl], scalar=c, in1=h_ap,
                        op0=AluOpType.mult, op1=AluOpType.subtract)
                return ev

            # =============== Phases A1 + A2 (merged psum scope) ===========
            qsbp_cm = tc.tile_pool(name="qsbp", bufs=2)
            qsbp = qsbp_cm.__enter__()
            bd01_cm = tc.tile_pool(name="bd01", bufs=1)
            bd01 = bd01_cm.__enter__()
            nei01 = bd01.tile([P, KC, 2, T // 2], F8, tag="nei01")
            ed01 = bd01.tile([P, KC, 2, T // 2], F8, tag="ed01")
            qsb_tiles = {}

            def load_qsb(sp):
                t = qsbp.tile([P, MB, 6, SPAN], F8, tag="qsb",
                              name=f"qsb{sp}")
                nc.sync.dma_start(
                    t[:], q[:, sp].rearrange("m p v t -> p m v t"))
                qsb_tiles[sp] = t

            with tc.tile_pool(name="eip", bufs=1) as eip, \
                 tc.tile_pool(name="esp", bufs=1) as esp:
                # ei/es are rolling 2-panel windows (A2+A1 consume jp-wise)
                eiw = [eip.tile([P, KC, 2, A1W], F8, tag=f"eiw{h}",
                                name=f"eiw{h}") for h in range(2)]
                esw = [esp.tile([P, KC, 2, A1W], F8, tag=f"esw{h}",
                                name=f"esw{h}") for h in range(2)]

                with tc.tile_pool(name="psA", bufs=2, space="PSUM") as psA, \
                     tc.tile_pool(name="rcA", bufs=2) as rcA, \
                     tc.tile_pool(name="evA", bufs=3) as evA, \
                     tc.tile_pool(name="pkA", bufs=2) as pkA, \
                     tc.tile_pool(name="wroll", bufs=2) as wrollp, \
                     tc.tile_pool(name="vres", bufs=1) as vres:
                    vr = vres.tile([P, KC, 2, D], F8, tag="vr")
                    vi = vres.tile([P, KC, 2, D], F8, tag="vi")
                    vs = vres.tile([P, KC, 2, D], F8, tag="vs")

                    w_tiles = {}

                    def load_w(key, m):
                        n = uid()
                        wrm = wrollp.tile([P, KC, 2, P], F8, tag="wr",
                                          name=f"wr{n}")
                        wim = wrollp.tile([P, KC, 2, P], F8, tag="wi",
                                          name=f"wi{n}")
                        wsm = wrollp.tile([P, KC, 2, P], F8, tag="ws",
                                          name=f"ws{n}")
                        nc.sync.dma_start(wrm[:], wp_r[m])
                        nc.sync.dma_start(wim[:], wp_i[m])
                        nc.sync.dma_start(wsm[:], wp_s[m])
                        w_tiles[key] = (wrm, wim, wsm)

                    def ewin_load(jp):
                        js = bass.ds(jp * A1W, A1W)
                        h = jp % 2
                        nc.sync.dma_start(eiw[h][:], ep_i[:, :, :, js])
                        nc.sync.dma_start(er[:, :, :, js], ep_r[:, :, :, js])
                        nc.sync.dma_start(esw[h][:], ep_s[:, :, :, js])

                    js0 = bass.ds(0, A1W)
                    nc.sync.dma_start(eiw[0][:], ep_i[:, :, :, js0])
                    nc.sync.dma_start(
                        vi[:], vp_i[:].rearrange("k p s m -> p k s m"))
                    nc.sync.dma_start(er[:, :, :, js0], ep_r[:, :, :, js0])
                    nc.sync.dma_start(
                        vr[:], vp_r[:].rearrange("k p s m -> p k s m"))
                    nc.sync.dma_start(esw[0][:], ep_s[:, :, :, js0])
                    nc.sync.dma_start(
                        vs[:], vp_s[:].rearrange("k p s m -> p k s m"))
                    vd = bass.ds(0, D)

                    def a2_site(tb, ei_t, es_t):
                        tbs = bass.ts(tb, P)
                        lsl = bass.ds((tb % 4) * P, P)
                        n = uid()
                        pp = psA.tile([P, 3, D], F32, tag="pp", name=f"pp{n}")
                        dr_product(nc, pp[:, 0], pack_h0(ei_t, lsl),
                                   pack_h1(vi, vd), KC)
                        dr_product(nc, pp[:, 1], pack_h0(er, tbs),
                                   pack_h1(vr, vd), KC)
                        dr_product(nc, pp[:, 2], pack_h0(es_t, lsl),
                                   pack_h1(vs, vd), KC)
                        ppk = pkA.tile([P, 6, D], F8, tag="pk",
                                       name=f"ppk{n}")
                        site_evac(pp, D, CQ, ppk[:, 1::2], ppk[:, 0::2],
                                  evA, rcA)
                        nc.sync.dma_start(pt[tb], ppk[:])

                    def a1_site(jp, m, ei_t, es_t):
                        js = bass.ds(jp * A1W, A1W)
                        fw = bass.ds(0, A1W)
                        fp128 = bass.ds(0, P)
                        wrm, wim, wsm = w_tiles.pop((jp, m))
                        n = uid()
                        pp = psA.tile([P, 3, A1W], F32, tag="pp",
                                      name=f"pp{n}")
                        dr_product(nc, pp[:, 0], pack_h1(wim, fp128),
                                   pack_h0(ei_t, fw), KC)
                        dr_product(nc, pp[:, 1], pack_h1(wrm, fp128),
                                   pack_h0(er, js), KC)
                        dr_product(nc, pp[:, 2], pack_h1(wsm, fp128),
                                   pack_h0(es_t, fw), KC)
                        qpk = pkA.tile([P, 2, 6, SPAN], F8, tag="qpk",
                                       name=f"qpk{n}")
                        pieces = [(qpk[:, h, 1::2, :], qpk[:, h, 0::2, :],
                                   slice(h * SPAN, (h + 1) * SPAN))
                                  for h in range(2)]
                        site_evac(pp, A1W, CQ, pieces, None, evA, rcA)
                        nc.sync.dma_start(q[m, 2 * jp], qpk[:, 0])
                        nc.sync.dma_start(q[m, 2 * jp + 1], qpk[:, 1])

                    pairs = [(jp, m) for jp in range(NJP) for m in range(MB)]
                    for jp in range(NJP):
                        ei_t, es_t = eiw[jp % 2], esw[jp % 2]
                        a2_site(4 * jp + 0, ei_t, es_t)
                        if jp == 0:
                            load_w((0, 0), 0)
                            load_w((0, 1), 1)
                        a2_site(4 * jp + 1, ei_t, es_t)
                        if jp == 0:
                            nc.sync.dma_start(mask_sb[:], trimask[:])
                            nc.sync.dma_start(rho_sb[:], rho2[:])
                        a2_site(4 * jp + 2, ei_t, es_t)
                        a2_site(4 * jp + 3, ei_t, es_t)
                        if jp + 1 < NJP:
                            ewin_load(jp + 1)
                        if jp in (1, 2):
                            hq = bass.ds((jp - 1) * A1W, A1W)
                            nc.sync.dma_start(nei01[:, :, :, hq],
                                              ep_ni[:, :, :, hq])
                            nc.sync.dma_start(ed01[:, :, :, hq],
                                              ep_d[:, :, :, hq])
                        for m in range(MB):
                            idx = jp * MB + m
                            if idx + 2 < len(pairs):
                                load_w(pairs[idx + 2], pairs[idx + 2][1])
                            a1_site(jp, m, ei_t, es_t)
                            if jp == 0 and m == MB - 1:
                                load_qsb(0)
                                load_qsb(1)

            # =============== Phase B: S = E^H Q (upper tri) ===============
            with tc.tile_pool(name="ptp", bufs=1) as ptpp:
                ptr = ptpp.tile([P, TB, 2, D], F8, tag="ptr")
                pti = ptpp.tile([P, TB, 2, D], F8, tag="pti")

                with tc.tile_pool(name="psB", bufs=3, space="PSUM") as psB, \
                     tc.tile_pool(name="rcB", bufs=3) as rcB, \
                     tc.tile_pool(name="evB", bufs=3) as evB, \
                     tc.tile_pool(name="spkp", bufs=3) as spkp, \
                     tc.tile_pool(name="ptsp", bufs=1) as ptsp:
                  pts = ptsp.tile([P, TB, 2, D], F8, tag="pts")
                  sst_small = {}
                  with tc.tile_pool(name="edp", bufs=1) as edp:
                    nei23 = edp.tile([P, KC, 2, T // 2], F8, tag="nei23")
                    ed23 = edp.tile([P, KC, 2, T // 2], F8, tag="ed23")
                    nc.sync.dma_start(
                        pts[:], pt[:, :, 4:6].rearrange("t p v d -> p t v d"))

                    def b_lhs(t01, t23, ib):
                        if ib < MB:
                            return pack_h0(t01, bass.ts(ib, P))
                        return pack_h0(t23, bass.ts(ib - MB, P))
                    nc.sync.dma_start(
                        ptr[:], pt[:, :, 0:2].rearrange("t p v d -> p t v d"))
                    nc.sync.dma_start(
                        pti[:], pt[:, :, 2:4].rearrange("t p v d -> p t v d"))

                    for sp in range(NSP):
                        if sp + 2 < NSP:
                            load_qsb(sp + 2)
                        if sp < 2:
                            lq = bass.ds(sp * A1W, A1W)
                            gq = bass.ds(T // 2 + sp * A1W, A1W)
                            nc.sync.dma_start(nei23[:, :, :, lq],
                                              ep_ni[:, :, :, gq])
                            nc.sync.dma_start(ed23[:, :, :, lq],
                                              ep_d[:, :, :, gq])
                        if sp == 2:
                            for _jb in range(2):
                                t = spkp.tile([P, 2, 6, P], F8, tag="sst_s",
                                              name=f"sst_s{_jb}")[:, :_jb + 1]
                                nc.sync.dma_start(
                                    t[:], s[:_jb + 1, _jb].rearrange(
                                        "i p v j -> p i v j"))
                                sst_small[_jb] = t
                        qsb = qsb_tiles.pop(sp)

                        def rhs_q(vb):
                            def f(k, kind):
                                if kind == "hh":
                                    return qsb[:, 2 * k:2 * k + 2, vb + 1, :]
                                return qsb[:, k, vb:vb + 2, :]
                            return f

                        for ib in range(2 * sp + 2):
                            ibs = bass.ts(ib, P)
                            top = ib == 2 * sp + 1  # low half would be garbage
                            w = P if top else SPAN

                            def rq(vb, _top=top):
                                base = rhs_q(vb)
                                if not _top:
                                    return base

                                def f(k, kind):
                                    return base(k, kind)[:, :, P:]
                                return f

                            n = uid()
                            pp = psB.tile([P, 3, SPAN], F32, tag="pp",
                                          name=f"pp{n}")[:, :, :w]
                            dr_product(nc, pp[:, 0], b_lhs(nei01, nei23, ib),
                                       rq(2), KC)
                            dr_product(nc, pp[:, 1], pack_h0(er, ibs),
                                       rq(0), KC)
                            dr_product(nc, pp[:, 2], b_lhs(ed01, ed23, ib),
                                       rq(4), KC)
                            masks = [bass.ds(0, P)] if (
                                top or ib == 2 * sp) else []
                            spk = spkp.tile([P, 2, 6, P], F8, tag="spk",
                                            name=f"spk{n}")
                            nh = 1 if top else 2
                            pieces = [(spk[:, jh, 0::2, :],
                                       spk[:, jh, 1::2, :],
                                       slice(jh * P, (jh + 1) * P))
                                      for jh in range(nh)]
                            site_evac(pp, w, CS, pieces, None,
                                      evB, rcB, masks=masks)
                            for jh in range(nh):
                                jb = 2 * sp + (1 if top else jh)
                                if ib <= jb:
                                    nc.sync.dma_start(s[ib, jb],
                                                      spk[:, jh])

                  # ======== Phase C (shares psB/rcB/evB pools) ========
                  with tc.tile_pool(name="sstp", bufs=2) as sstp, \
                       tc.tile_pool(name="out4", bufs=3) as out4:
                    sst_tiles = {}

                    def load_sst(jb):
                        t = sstp.tile([P, TB, 6, P], F8, tag="sst",
                                      name=f"sst{jb}")[:, :jb + 1]
                        nc.sync.dma_start(
                            t[:], s[:jb + 1, jb].rearrange(
                                "i p v j -> p i v j"))
                        sst_tiles[jb] = t

                    sst_tiles.update(sst_small)
                    load_sst(2)
                    for jb in range(TB):
                        jbs = bass.ts(jb, P)
                        nk = jb + 1
                        if 2 <= jb + 1 < TB:
                            load_sst(jb + 1)
                        sst = sst_tiles.pop(jb)

                        def lhs_s(vb):
                            def f(k, kind):
                                if kind == "hh":
                                    return sst[:, 2 * k:2 * k + 2, vb, :]
                                return sst[:, k, vb:vb + 2, :]
                            return f

                        kl = nk - 1
                        oo = out4.tile([P, 2, D], F32, tag="oo",
                                       name=f"oo{jb}")
                        for ch in range(2):
                            cds = bass.ds(ch * SPAN, SPAN)
                            n = uid()
                            pp = psB.tile([P, 3, SPAN], F32, tag="pp",
                                          name=f"pp{n}")
                            dr_product(nc, pp[:, 0], lhs_s(2),
                                       pack_h1(pti, cds), nk,
                                       leftover=(sst[:, kl, 2, :],
                                                 pti[:, kl, 1, cds]))
                            dr_product(nc, pp[:, 1], lhs_s(0),
                                       pack_h1(ptr, cds), nk,
                                       leftover=(sst[:, kl, 0, :],
                                                 ptr[:, kl, 1, cds]))
                            dr_product(nc, pp[:, 2], lhs_s(4),
                                       pack_h1(pts, cds), nk,
                                       leftover=(sst[:, kl, 4, :],
                                                 pts[:, kl, 1, cds]))
                            ev = site_evac(pp, SPAN, None, None, None,
                                           evB, rcB)
                            nc.scalar.activation(
                                oo[:, :, cds], ev[:, 0:2], COPY,
                                scale=rho_sb[:, jb:jb + 1])
                        nc.sync.dma_start(outT_re[jbs, :], oo[:, 0])
                        nc.sync.dma_start(outT_im[jbs, :], oo[:, 1])
            bd01_cm.__exit__(None, None, None)
            qsbp_cm.__exit__(None, None, None)

    nc.compile()
    return nc


_NC_CACHE = None


def _get_module():
    global _NC_CACHE
    if _NC_CACHE is None:
        _NC_CACHE = build_module()
    return _NC_CACHE


def _split(x):
    h = x.astype(E4NP)
    l = (x - h.astype(np.float32)).astype(E4NP)
    return h, l


def _pack(x, hfirst):
    """x [D2, F] f32 -> fp8 pack: [P, KC, 2, F] (E, h-first) or
    [KC, P, 2, F] (weights, l-first)."""
    h, l = _split(x)
    F = x.shape[1]
    if hfirst:
        out = np.empty((P, KC, 2, F), E4NP)
        out[:, :, 0] = h.reshape(KC, P, F).transpose(1, 0, 2)
        out[:, :, 1] = l.reshape(KC, P, F).transpose(1, 0, 2)
    else:
        out = np.empty((KC, P, 2, F), E4NP)
        out[:, :, 1] = h.reshape(KC, P, F)
        out[:, :, 0] = l.reshape(KC, P, F)
    return out


def _pack_w(w):
    """w [D2, D2] (c, m) f32 -> [MB, P, KC, 2(l,h), P] fp8 pack."""
    h, l = _split(w)
    out = np.empty((MB, P, KC, 2, P), E4NP)
    out[:, :, :, 1] = h.reshape(KC, P, MB, P).transpose(2, 1, 0, 3)
    out[:, :, :, 0] = l.reshape(KC, P, MB, P).transpose(2, 1, 0, 3)
    return out


def prep_shared(WKQ_re, WKQ_im, WPV_re, WPV_im):
    wr = np.ascontiguousarray(WKQ_re.T) * 256.0
    wi = np.ascontiguousarray(WKQ_im.T) * 256.0
    vr = np.ascontiguousarray(WPV_re.T) * 256.0
    vi = np.ascontiguousarray(WPV_im.T) * 256.0
    shared = {
        "wp_r": _pack_w(wr), "wp_i": _pack_w(wi),
        "wp_s": _pack_w(wr + wi),
        "vp_r": _pack(vr, False), "vp_i": _pack(vi, False),
        "vp_s": _pack(vr + vi, False),
        "trimask": np.triu(np.ones((P, P), np.float32)),
    }
    j = np.arange(T, dtype=np.float32)
    rho = 2.0 / np.maximum(j, 1.0)
    shared["rho2"] = np.ascontiguousarray(rho.reshape(TB, P).T)
    return shared


def kernel(E_re, E_im, WKQ_re, WKQ_im, WPV_re, WPV_im):
    E_re = np.asarray(E_re, dtype=np.float32)
    E_im = np.asarray(E_im, dtype=np.float32)
    shared = prep_shared(np.asarray(WKQ_re, np.float32),
                         np.asarray(WKQ_im, np.float32),
                         np.asarray(WPV_re, np.float32),
                         np.asarray(WPV_im, np.float32))
    in_maps = []
    for b in range(B):
        er = E_re[b] * 4.0
        ei = E_im[b] * 4.0
        m = dict(shared)
        m["ep_r"] = _pack(er, True)
        m["ep_i"] = _pack(ei, True)
        m["ep_ni"] = _pack(-ei, True)
        m["ep_s"] = _pack(er + ei, True)
        m["ep_d"] = _pack(er - ei, True)
        in_maps.append(m)

    nc = _get_module()
    res = run_bass_kernel_spmd(nc, in_maps, core_ids=list(range(B)))

    out = np.empty((B, D, T - 2), dtype=np.complex64)
    for b in range(B):
        r = res.results[b]["outT_re"]  # [T, D]
        i = res.results[b]["outT_im"]
        full = (r + 1j * i.astype(np.complex64)).T  # [D, T]
        out[b] = full[:, 1:T - 1]
    return out


# revision 40
# speedup vs baseline: 1.0589x; 1.0181x over previous
"""Trainium2 Bass kernel for nn_AutoregressiveLSA — fp8 DoubleRow version.

Math (complex, per batch b, one NeuronCore per batch element):
    Q  = WKQ @ E                       [2d, T]
    S  = E^H @ Q, keep i <= j          [T, T]
    outT[j] = sum_{i<=j} S[i,j] PT[i] * 2/max(j,1),  PT = (WPV @ E)^T

All matmuls run as fp8e4 (e4m3) in DoubleRow perf mode: one PE
instruction contracts TWO 128-chunks at 0.5 cycles/output-column (4x
the fp32r MAC rate).  Precision comes from a hi/lo split of every
operand (x ~ x_h + x_l, both e4m3; x_l*y_l dropped): per 128-chunk each
real product needs 3 fp8 pairings = 1.5 DR instructions, so a complex
Karatsuba product costs 2.25 free-columns/chunk vs 3.0 for fp32r.
Measured end-to-end rel err ~3e-3 (gate 2e-2).

Scale chain (powers of 2, folded into casts / final rho):
    E*4, WKQ^T*256, WPV^T*256 quantized on host.
    A1 psum = 1024*Q,  split scale 2^-7  -> Q'' = 8Q
    A2 psum = 1024*PT, split scale 2^-7  -> PT'' = 8PT
    B  psum = 32*S,    split scale 2^-9  -> S'' = S/16
    C  psum = S*PT/2,  rho2 = 2/max(j,1) applied via Act scale.

Engine constraints honored (probed on real TRN2): vector ops may read
at most ONE psum operand; Pool (gpsimd) runs SBUF-only tensor_tensor
(no psum, no scalar_tensor_tensor); Act does scaled copies (fp8 out ok).
Evacuation is fused into wide ops: psum banks ordered (M2, M1, M3) so
one 3W psum->sbuf copy + one dual-sub [re,tt] + pool im/sum + ONE 3W
Act h-cast + ONE 3W DVE stt l-split handle a whole complex site.
Phase B uses a host-negated E_im pack (nei) so its conjugated
recombination has the same (M1-M2', M3-M1-M2') form as the others.
"""

import numpy as np
import ml_dtypes

import concourse.bass as bass
import concourse.mybir as mybir
import concourse.tile as tile
from concourse import bacc
from concourse.bass_utils import run_bass_kernel_spmd
from concourse.alu_op_type import AluOpType

F32 = mybir.dt.float32
F8 = mybir.dt.float8e4
E4NP = ml_dtypes.float8_e4m3
DR = mybir.MatmulPerfMode.DoubleRow
COPY = mybir.ActivationFunctionType.Copy

B = 8
D2 = 1024
T = 2048
D = 512
P = 128
KC = D2 // P
MB = D2 // P
TB = T // P
A1W = 512
NJP = T // A1W
SPAN = 256
NSP = T // SPAN

CQ = float(2.0 ** -7)
CS = float(2.0 ** -9)


def pack_h0(t, fsl):
    """Slicer for h-first packs [P, K, 2(h,l), F] (E/S side)."""
    def f(k, kind):
        if kind == "hh":
            return t[:, 2 * k:2 * k + 2, 0, fsl]
        return t[:, k, :, fsl]
    return f


def pack_h1(t, fsl):
    """Slicer for l-first packs [P, K, 2(l,h), F] (W/Q/PT side)."""
    def f(k, kind):
        if kind == "hh":
            return t[:, 2 * k:2 * k + 2, 1, fsl]
        return t[:, k, :, fsl]
    return f


def dr_product(nc, bank, lhs, rhs, nk, leftover=None):
    nhh = nk // 2
    odd = nk % 2
    tot = nhh + nk + (1 if odd else 0)
    i = 0
    for kp in range(nhh):
        nc.tensor.matmul(bank, lhs(kp, "hh"), rhs(kp, "hh"),
                         start=(i == 0), stop=(i == tot - 1), perf_mode=DR)
        i += 1
    for k in range(nk):
        nc.tensor.matmul(bank, lhs(k, "x"), rhs(k, "x"),
                         start=(i == 0), stop=(i == tot - 1), perf_mode=DR)
        i += 1
    if odd:
        la, ra = leftover
        nc.tensor.matmul(bank, la, ra, start=(i == 0), stop=(i == tot - 1))


def build_module():
    nc = bacc.Bacc(target_bir_lowering=False, trn_type="TRN2")

    ep_r = nc.dram_tensor("ep_r", [P, KC, 2, T], F8, kind="ExternalInput")
    ep_i = nc.dram_tensor("ep_i", [P, KC, 2, T], F8, kind="ExternalInput")
    ep_ni = nc.dram_tensor("ep_ni", [P, KC, 2, T], F8, kind="ExternalInput")
    ep_s = nc.dram_tensor("ep_s", [P, KC, 2, T], F8, kind="ExternalInput")
    ep_d = nc.dram_tensor("ep_d", [P, KC, 2, T], F8, kind="ExternalInput")
    wp_r = nc.dram_tensor("wp_r", [MB, P, KC, 2, P], F8, kind="ExternalInput")
    wp_i = nc.dram_tensor("wp_i", [MB, P, KC, 2, P], F8, kind="ExternalInput")
    wp_s = nc.dram_tensor("wp_s", [MB, P, KC, 2, P], F8, kind="ExternalInput")
    vp_r = nc.dram_tensor("vp_r", [KC, P, 2, D], F8, kind="ExternalInput")
    vp_i = nc.dram_tensor("vp_i", [KC, P, 2, D], F8, kind="ExternalInput")
    vp_s = nc.dram_tensor("vp_s", [KC, P, 2, D], F8, kind="ExternalInput")
    trimask = nc.dram_tensor("trimask", [P, P], F32, kind="ExternalInput")
    rho2 = nc.dram_tensor("rho2", [P, TB], F32, kind="ExternalInput")
    outT_re = nc.dram_tensor("outT_re", [T, D], F32, kind="ExternalOutput")
    outT_im = nc.dram_tensor("outT_im", [T, D], F32, kind="ExternalOutput")

    _n = [0]

    def uid():
        _n[0] += 1
        return _n[0]

    with tile.TileContext(nc) as tc:
        with tc.tile_pool(name="dram", bufs=1, space="DRAM") as dram, \
             tc.tile_pool(name="erp", bufs=1) as erp, \
             tc.tile_pool(name="cst", bufs=1) as cst:
            q = dram.tile([MB, NSP, P, 6, SPAN], F8, tag="q")
            pt = dram.tile([TB, P, 6, D], F8, tag="pt")
            s = dram.tile([TB, TB, P, 6, P], F8, tag="s")

            er = erp.tile([P, KC, 2, T], F8, tag="er")
            mask_sb = cst.tile([P, P], F32, tag="mask")
            rho_sb = cst.tile([P, TB], F32, tag="rho")

            def site_evac(pp, width, c, pk_h_ap, pk_l_ap, ev_pool, rc_pool,
                          masks=None):
                """Evacuate one complex site.

                pp: psum tile [P, 3, width] with banks (M2, M1, M3).
                pk_h_ap/pk_l_ap: output APs for h/l fp8 splits of
                (re, im, sum), or None to skip splits (phase C).
                Returns ev tile [P, 4, width] = (re, im, sum, tt).
                """
                n = uid()
                rc = rc_pool.tile([P, 3, width], F32, tag="rc", name=f"rc{n}")
                ev = ev_pool.tile([P, 4, width], F32, tag="ev", name=f"ev{n}")
                nc.scalar.activation(rc[:], pp[:], COPY)
                nc.vector.tensor_sub(ev[:, 0::3], rc[:, 1:3], rc[:, 0:2])
                nc.gpsimd.tensor_sub(ev[:, 1], ev[:, 3], rc[:, 0])
                if masks is not None:
                    for dsl in masks:
                        nc.vector.tensor_mul(ev[:, 0, dsl], ev[:, 0, dsl],
                                             mask_sb[:])
                        nc.vector.tensor_mul(ev[:, 1, dsl], ev[:, 1, dsl],
                                             mask_sb[:])
                if pk_h_ap is None:
                    return ev
                nc.gpsimd.tensor_add(ev[:, 2], ev[:, 0], ev[:, 1])
                pieces = pk_h_ap if isinstance(pk_h_ap, list) \
                    else [(pk_h_ap, pk_l_ap, slice(None))]
                for h_ap, l_ap, csl in pieces:
                    nc.scalar.activation(h_ap, ev[:, 0:3, csl], COPY, scale=c)
                    nc.vector.scalar_tensor_tensor(
                        out=l_ap, in0=ev[:, 0:3, csl], scalar=c, in1=h_ap,
                        op0=AluOpType.mult, op1=AluOpType.subtract)
                return ev

            # =============== Phases A1 + A2 (merged psum scope) ===========
            qsbp_cm = tc.tile_pool(name="qsbp", bufs=2)
            qsbp = qsbp_cm.__enter__()
            bd01_cm = tc.tile_pool(name="bd01", bufs=1)
            bd01 = bd01_cm.__enter__()
            nei01 = bd01.tile([P, KC, 2, T // 2], F8, tag="nei01")
            ed01 = bd01.tile([P, KC, 2, T // 2], F8, tag="ed01")
            qsb_tiles = {}

            def load_qsb(sp):
                t = qsbp.tile([P, MB, 6, SPAN], F8, tag="qsb",
                              name=f"qsb{sp}")
                nc.sync.dma_start(
                    t[:], q[:, sp].rearrange("m p v t -> p m v t"))
                qsb_tiles[sp] = t

            with tc.tile_pool(name="eip", bufs=1) as eip, \
                 tc.tile_pool(name="esp", bufs=1) as esp:
                # ei/es are rolling 2-panel windows (A2+A1 consume jp-wise)
                eiw = [eip.tile([P, KC, 2, A1W], F8, tag=f"eiw{h}",
                                name=f"eiw{h}") for h in range(2)]
                esw = [esp.tile([P, KC, 2, A1W], F8, tag=f"esw{h}",
                                name=f"esw{h}") for h in range(2)]

                with tc.tile_pool(name="psA", bufs=2, space="PSUM") as psA, \
                     tc.tile_pool(name="rcA", bufs=2) as rcA, \
                     tc.tile_pool(name="evA", bufs=2) as evA, \
                     tc.tile_pool(name="pkA", bufs=3) as pkA, \
                     tc.tile_pool(name="wroll", bufs=2) as wrollp, \
                     tc.tile_pool(name="vres", bufs=1) as vres:
                    vr = vres.tile([P, KC, 2, D], F8, tag="vr")
                    vi = vres.tile([P, KC, 2, D], F8, tag="vi")
                    vs = vres.tile([P, KC, 2, D], F8, tag="vs")

                    w_tiles = {}

                    def load_w(key, m):
                        n = uid()
                        wrm = wrollp.tile([P, KC, 2, P], F8, tag="wr",
                                          name=f"wr{n}")
                        wim = wrollp.tile([P, KC, 2, P], F8, tag="wi",
                                          name=f"wi{n}")
                        wsm = wrollp.tile([P, KC, 2, P], F8, tag="ws",
                                          name=f"ws{n}")
                        nc.sync.dma_start(wrm[:], wp_r[m])
                        nc.sync.dma_start(wim[:], wp_i[m])
                        nc.sync.dma_start(wsm[:], wp_s[m])
                        w_tiles[key] = (wrm, wim, wsm)

                    def ewin_load(jp):
                        js = bass.ds(jp * A1W, A1W)
                        h = jp % 2
                        nc.sync.dma_start(eiw[h][:], ep_i[:, :, :, js])
                        nc.sync.dma_start(er[:, :, :, js], ep_r[:, :, :, js])
                        nc.sync.dma_start(esw[h][:], ep_s[:, :, :, js])

                    js0 = bass.ds(0, A1W)
                    nc.sync.dma_start(eiw[0][:], ep_i[:, :, :, js0])
                    nc.sync.dma_start(
                        vi[:], vp_i[:].rearrange("k p s m -> p k s m"))
                    nc.sync.dma_start(er[:, :, :, js0], ep_r[:, :, :, js0])
                    nc.sync.dma_start(
                        vr[:], vp_r[:].rearrange("k p s m -> p k s m"))
                    nc.sync.dma_start(esw[0][:], ep_s[:, :, :, js0])
                    nc.sync.dma_start(
                        vs[:], vp_s[:].rearrange("k p s m -> p k s m"))
                    vd = bass.ds(0, D)

                    def a2_site(tb, ei_t, es_t):
                        tbs = bass.ts(tb, P)
                        lsl = bass.ds((tb % 4) * P, P)
                        n = uid()
                        pp = psA.tile([P, 3, D], F32, tag="pp", name=f"pp{n}")
                        dr_product(nc, pp[:, 0], pack_h0(ei_t, lsl),
                                   pack_h1(vi, vd), KC)
                        dr_product(nc, pp[:, 1], pack_h0(er, tbs),
                                   pack_h1(vr, vd), KC)
                        dr_product(nc, pp[:, 2], pack_h0(es_t, lsl),
                                   pack_h1(vs, vd), KC)
                        ppk = pkA.tile([P, 6, D], F8, tag="pk",
                                       name=f"ppk{n}")
                        site_evac(pp, D, CQ, ppk[:, 1::2], ppk[:, 0::2],
                                  evA, rcA)
                        nc.sync.dma_start(pt[tb], ppk[:])

                    def a1_site(jp, m, ei_t, es_t):
                        js = bass.ds(jp * A1W, A1W)
                        fw = bass.ds(0, A1W)
                        fp128 = bass.ds(0, P)
                        wrm, wim, wsm = w_tiles.pop((jp, m))
                        n = uid()
                        pp = psA.tile([P, 3, A1W], F32, tag="pp",
                                      name=f"pp{n}")
                        dr_product(nc, pp[:, 0], pack_h1(wim, fp128),
                                   pack_h0(ei_t, fw), KC)
                        dr_product(nc, pp[:, 1], pack_h1(wrm, fp128),
                                   pack_h0(er, js), KC)
                        dr_product(nc, pp[:, 2], pack_h1(wsm, fp128),
                                   pack_h0(es_t, fw), KC)
                        qpk = pkA.tile([P, 2, 6, SPAN], F8, tag="qpk",
                                       name=f"qpk{n}")
                        pieces = [(qpk[:, h, 1::2, :], qpk[:, h, 0::2, :],
                                   slice(h * SPAN, (h + 1) * SPAN))
                                  for h in range(2)]
                        site_evac(pp, A1W, CQ, pieces, None, evA, rcA)
                        nc.sync.dma_start(q[m, 2 * jp], qpk[:, 0])
                        nc.sync.dma_start(q[m, 2 * jp + 1], qpk[:, 1])

                    pairs = [(jp, m) for jp in range(NJP) for m in range(MB)]
                    for jp in range(NJP):
                        ei_t, es_t = eiw[jp % 2], esw[jp % 2]
                        a2_site(4 * jp + 0, ei_t, es_t)
                        if jp == 0:
                            load_w((0, 0), 0)
                            load_w((0, 1), 1)
                        a2_site(4 * jp + 1, ei_t, es_t)
                        if jp == 0:
                            nc.sync.dma_start(mask_sb[:], trimask[:])
                            nc.sync.dma_start(rho_sb[:], rho2[:])
                        a2_site(4 * jp + 2, ei_t, es_t)
                        a2_site(4 * jp + 3, ei_t, es_t)
                        if jp + 1 < NJP:
                            ewin_load(jp + 1)
                        if jp in (1, 2):
                            hq = bass.ds((jp - 1) * A1W, A1W)
                            nc.sync.dma_start(nei01[:, :, :, hq],
                                              ep_ni[:, :, :, hq])
                            nc.sync.dma_start(ed01[:, :, :, hq],
                                              ep_d[:, :, :, hq])
                        for m in range(MB):
                            idx = jp * MB + m
                            if idx + 2 < len(pairs):
                                load_w(pairs[idx + 2], pairs[idx + 2][1])
                            a1_site(jp, m, ei_t, es_t)
                            if jp == 0 and m == MB - 1:
                                load_qsb(0)
                                load_qsb(1)

            # =============== Phase B: S = E^H Q (upper tri) ===============
            with tc.tile_pool(name="ptp", bufs=1) as ptpp:
                ptr = ptpp.tile([P, TB, 2, D], F8, tag="ptr")
                pti = ptpp.tile([P, TB, 2, D], F8, tag="pti")

                with tc.tile_pool(name="psB", bufs=3, space="PSUM") as psB, \
                     tc.tile_pool(name="rcB", bufs=3) as rcB, \
                     tc.tile_pool(name="evB", bufs=3) as evB, \
                     tc.tile_pool(name="spkp", bufs=3) as spkp, \
                     tc.tile_pool(name="ptsp", bufs=1) as ptsp:
                  pts = ptsp.tile([P, TB, 2, D], F8, tag="pts")
                  sst_small = {}
                  with tc.tile_pool(name="edp", bufs=1) as edp:
                    nei23 = edp.tile([P, KC, 2, T // 2], F8, tag="nei23")
                    ed23 = edp.tile([P, KC, 2, T // 2], F8, tag="ed23")
                    nc.sync.dma_start(
                        pts[:], pt[:, :, 4:6].rearrange("t p v d -> p t v d"))

                    def b_lhs(t01, t23, ib):
                        if ib < MB:
                            return pack_h0(t01, bass.ts(ib, P))
                        return pack_h0(t23, bass.ts(ib - MB, P))
                    nc.sync.dma_start(
                        ptr[:], pt[:, :, 0:2].rearrange("t p v d -> p t v d"))
                    nc.sync.dma_start(
                        pti[:], pt[:, :, 2:4].rearrange("t p v d -> p t v d"))

                    for sp in range(NSP):
                        if sp + 2 < NSP:
                            load_qsb(sp + 2)
                        if sp < 2:
                            lq = bass.ds(sp * A1W, A1W)
                            gq = bass.ds(T // 2 + sp * A1W, A1W)
                            nc.sync.dma_start(nei23[:, :, :, lq],
                                              ep_ni[:, :, :, gq])
                            nc.sync.dma_start(ed23[:, :, :, lq],
                                              ep_d[:, :, :, gq])
                        if sp == 2:
                            for _jb in range(2):
                                t = spkp.tile([P, 2, 6, P], F8, tag="sst_s",
                                              name=f"sst_s{_jb}")[:, :_jb + 1]
                                nc.sync.dma_start(
                                    t[:], s[:_jb + 1, _jb].rearrange(
                                        "i p v j -> p i v j"))
                                sst_small[_jb] = t
                        qsb = qsb_tiles.pop(sp)

                        def rhs_q(vb):
                            def f(k, kind):
                                if kind == "hh":
                                    return qsb[:, 2 * k:2 * k + 2, vb + 1, :]
                                return qsb[:, k, vb:vb + 2, :]
                            return f

                        for ib in range(2 * sp + 2):
                            ibs = bass.ts(ib, P)
                            top = ib == 2 * sp + 1  # low half would be garbage
                            w = P if top else SPAN

                            def rq(vb, _top=top):
                                base = rhs_q(vb)
                                if not _top:
                                    return base

                                def f(k, kind):
                                    return base(k, kind)[:, :, P:]
                                return f

                            n = uid()
                            pp = psB.tile([P, 3, SPAN], F32, tag="pp",
                                          name=f"pp{n}")[:, :, :w]
                            dr_product(nc, pp[:, 0], b_lhs(nei01, nei23, ib),
                                       rq(2), KC)
                            dr_product(nc, pp[:, 1], pack_h0(er, ibs),
                                       rq(0), KC)
                            dr_product(nc, pp[:, 2], b_lhs(ed01, ed23, ib),
                                       rq(4), KC)
                            masks = [bass.ds(0, P)] if (
                                top or ib == 2 * sp) else []
                            spk = spkp.tile([P, 2, 6, P], F8, tag="spk",
                                            name=f"spk{n}")
                            nh = 1 if top else 2
                            pieces = [(spk[:, jh, 0::2, :],
                                       spk[:, jh, 1::2, :],
                                       slice(jh * P, (jh + 1) * P))
                                      for jh in range(nh)]
                            site_evac(pp, w, CS, pieces, None,
                                      evB, rcB, masks=masks)
                            for jh in range(nh):
                                jb = 2 * sp + (1 if top else jh)
                                if ib <= jb:
                                    nc.sync.dma_start(s[ib, jb],
                                                      spk[:, jh])

                  # ======== Phase C (shares psB/rcB/evB pools) ========
                  with tc.tile_pool(name="sstp", bufs=2) as sstp, \
                       tc.tile_pool(name="out4", bufs=3) as out4:
                    sst_tiles = {}

                    def load_sst(jb):
                        t = sstp.tile([P, TB, 6, P], F8, tag="sst",
                                      name=f"sst{jb}")[:, :jb + 1]
                        nc.sync.dma_start(
                            t[:], s[:jb + 1, jb].rearrange(
                                "i p v j -> p i v j"))
                        sst_tiles[jb] = t

                    sst_tiles.update(sst_small)
                    load_sst(2)
                    for jb in range(TB):
                        jbs = bass.ts(jb, P)
                        nk = jb + 1
                        if 2 <= jb + 1 < TB:
                            load_sst(jb + 1)
                        sst = sst_tiles.pop(jb)

                        def lhs_s(vb):
                            def f(k, kind):
                                if kind == "hh":
                                    return sst[:, 2 * k:2 * k + 2, vb, :]
                                return sst[:, k, vb:vb + 2, :]
                            return f

                        kl = nk - 1
                        oo = out4.tile([P, 2, D], F32, tag="oo",
                                       name=f"oo{jb}")
                        for ch in range(2):
                            cds = bass.ds(ch * SPAN, SPAN)
                            n = uid()
                            pp = psB.tile([P, 3, SPAN], F32, tag="pp",
                                          name=f"pp{n}")
                            dr_product(nc, pp[:, 0], lhs_s(2),
                                       pack_h1(pti, cds), nk,
                                       leftover=(sst[:, kl, 2, :],
                                                 pti[:, kl, 1, cds]))
                            dr_product(nc, pp[:, 1], lhs_s(0),
                                       pack_h1(ptr, cds), nk,
                                       leftover=(sst[:, kl, 0, :],
                                                 ptr[:, kl, 1, cds]))
                            dr_product(nc, pp[:, 2], lhs_s(4),
                                       pack_h1(pts, cds), nk,
                                       leftover=(sst[:, kl, 4, :],
                                                 pts[:, kl, 1, cds]))
                            ev = site_evac(pp, SPAN, None, None, None,
                                           evB, rcB)
                            nc.scalar.activation(
                                oo[:, :, cds], ev[:, 0:2], COPY,
                                scale=rho_sb[:, jb:jb + 1])
                        nc.sync.dma_start(outT_re[jbs, :], oo[:, 0])
                        nc.sync.dma_start(outT_im[jbs, :], oo[:, 1])
            bd01_cm.__exit__(None, None, None)
            qsbp_cm.__exit__(None, None, None)

    nc.compile()
    return nc


_NC_CACHE = None


def _get_module():
    global _NC_CACHE
    if _NC_CACHE is None:
        _NC_CACHE = build_module()
    return _NC_CACHE


def _split(x):
    h = x.astype(E4NP)
    l = (x - h.astype(np.float32)).astype(E4NP)
    return h, l


def _pack(x, hfirst):
    """x [D2, F] f32 -> fp8 pack: [P, KC, 2, F] (E, h-first) or
    [KC, P, 2, F] (weights, l-first)."""
    h, l = _split(x)
    F = x.shape[1]
    if hfirst:
        out = np.empty((P, KC, 2, F), E4NP)
        out[:, :, 0] = h.reshape(KC, P, F).transpose(1, 0, 2)
        out[:, :, 1] = l.reshape(KC, P, F).transpose(1, 0, 2)
    else:
        out = np.empty((KC, P, 2, F), E4NP)
        out[:, :, 1] = h.reshape(KC, P, F)
        out[:, :, 0] = l.reshape(KC, P, F)
    return out


def _pack_w(w):
    """w [D2, D2] (c, m) f32 -> [MB, P, KC, 2(l,h), P] fp8 pack."""
    h, l = _split(w)
    out = np.empty((MB, P, KC, 2, P), E4NP)
    out[:, :, :, 1] = h.reshape(KC, P, MB, P).transpose(2, 1, 0, 3)
    out[:, :, :, 0] = l.reshape(KC, P, MB, P).transpose(2, 1, 0, 3)
    return out


def prep_shared(WKQ_re, WKQ_im, WPV_re, WPV_im):
    wr = np.ascontiguousarray(WKQ_re.T) * 256.0
    wi = np.ascontiguousarray(WKQ_im.T) * 256.0
    vr = np.ascontiguousarray(WPV_re.T) * 256.0
    vi = np.ascontiguousarray(WPV_im.T) * 256.0
    shared = {
        "wp_r": _pack_w(wr), "wp_i": _pack_w(wi),
        "wp_s": _pack_w(wr + wi),
        "vp_r": _pack(vr, False), "vp_i": _pack(vi, False),
        "vp_s": _pack(vr + vi, False),
        "trimask": np.triu(np.ones((P, P), np.float32)),
    }
    j = np.arange(T, dtype=np.float32)
    rho = 2.0 / np.maximum(j, 1.0)
    shared["rho2"] = np.ascontiguousarray(rho.reshape(TB, P).T)
    return shared


def kernel(E_re, E_im, WKQ_re, WKQ_im, WPV_re, WPV_im):
    E_re = np.asarray(E_re, dtype=np.float32)
    E_im = np.asarray(E_im, dtype=np.float32)
    shared = prep_shared(np.asarray(WKQ_re, np.float32),
                         np.asarray(WKQ_im, np.float32),
                         np.asarray(WPV_re, np.float32),
                         np.asarray(WPV_im, np.float32))
    in_maps = []
    for b in range(B):
        er = E_re[b] * 4.0
        ei = E_im[b] * 4.0
        m = dict(shared)
        m["ep_r"] = _pack(er, True)
        m["ep_i"] = _pack(ei, True)
        m["ep_ni"] = _pack(-ei, True)
        m["ep_s"] = _pack(er + ei, True)
        m["ep_d"] = _pack(er - ei, True)
        in_maps.append(m)

    nc = _get_module()
    res = run_bass_kernel_spmd(nc, in_maps, core_ids=list(range(B)))

    out = np.empty((B, D, T - 2), dtype=np.complex64)
    for b in range(B):
        r = res.results[b]["outT_re"]  # [T, D]
        i = res.results[b]["outT_im"]
        full = (r + 1j * i.astype(np.complex64)).T  # [D, T]
        out[b] = full[:, 1:T - 1]
    return out


# revision 41
# speedup vs baseline: 1.0669x; 1.0076x over previous
"""Trainium2 Bass kernel for nn_AutoregressiveLSA — fp8 DoubleRow version.

Math (complex, per batch b, one NeuronCore per batch element):
    Q  = WKQ @ E                       [2d, T]
    S  = E^H @ Q, keep i <= j          [T, T]
    outT[j] = sum_{i<=j} S[i,j] PT[i] * 2/max(j,1),  PT = (WPV @ E)^T

All matmuls run as fp8e4 (e4m3) in DoubleRow perf mode: one PE
instruction contracts TWO 128-chunks at 0.5 cycles/output-column (4x
the fp32r MAC rate).  Precision comes from a hi/lo split of every
operand (x ~ x_h + x_l, both e4m3; x_l*y_l dropped): per 128-chunk each
real product needs 3 fp8 pairings = 1.5 DR instructions, so a complex
Karatsuba product costs 2.25 free-columns/chunk vs 3.0 for fp32r.
Measured end-to-end rel err ~3e-3 (gate 2e-2).

Scale chain (powers of 2, folded into casts / final rho):
    E*4, WKQ^T*256, WPV^T*256 quantized on host.
    A1 psum = 1024*Q,  split scale 2^-7  -> Q'' = 8Q
    A2 psum = 1024*PT, split scale 2^-7  -> PT'' = 8PT
    B  psum = 32*S,    split scale 2^-9  -> S'' = S/16
    C  psum = S*PT/2,  rho2 = 2/max(j,1) applied via Act scale.

Engine constraints honored (probed on real TRN2): vector ops may read
at most ONE psum operand; Pool (gpsimd) runs SBUF-only tensor_tensor
(no psum, no scalar_tensor_tensor); Act does scaled copies (fp8 out ok).
Evacuation is fused into wide ops: psum banks ordered (M2, M1, M3) so
one 3W psum->sbuf copy + one dual-sub [re,tt] + pool im/sum + ONE 3W
Act h-cast + ONE 3W DVE stt l-split handle a whole complex site.
Phase B uses a host-negated E_im pack (nei) so its conjugated
recombination has the same (M1-M2', M3-M1-M2') form as the others.
"""

import numpy as np
import ml_dtypes

import concourse.bass as bass
import concourse.mybir as mybir
import concourse.tile as tile
from concourse import bacc
from concourse.bass_utils import run_bass_kernel_spmd
from concourse.alu_op_type import AluOpType

F32 = mybir.dt.float32
F8 = mybir.dt.float8e4
E4NP = ml_dtypes.float8_e4m3
DR = mybir.MatmulPerfMode.DoubleRow
COPY = mybir.ActivationFunctionType.Copy

B = 8
D2 = 1024
T = 2048
D = 512
P = 128
KC = D2 // P
MB = D2 // P
TB = T // P
A1W = 512
NJP = T // A1W
SPAN = 256
NSP = T // SPAN

CQ = float(2.0 ** -7)
CS = float(2.0 ** -9)


def pack_h0(t, fsl):
    """Slicer for h-first packs [P, K, 2(h,l), F] (E/S side)."""
    def f(k, kind):
        if kind == "hh":
            return t[:, 2 * k:2 * k + 2, 0, fsl]
        return t[:, k, :, fsl]
    return f


def pack_h1(t, fsl):
    """Slicer for l-first packs [P, K, 2(l,h), F] (W/Q/PT side)."""
    def f(k, kind):
        if kind == "hh":
            return t[:, 2 * k:2 * k + 2, 1, fsl]
        return t[:, k, :, fsl]
    return f


def dr_product(nc, bank, lhs, rhs, nk, leftover=None):
    nhh = nk // 2
    odd = nk % 2
    tot = nhh + nk + (1 if odd else 0)
    i = 0
    for kp in range(nhh):
        nc.tensor.matmul(bank, lhs(kp, "hh"), rhs(kp, "hh"),
                         start=(i == 0), stop=(i == tot - 1), perf_mode=DR)
        i += 1
    for k in range(nk):
        nc.tensor.matmul(bank, lhs(k, "x"), rhs(k, "x"),
                         start=(i == 0), stop=(i == tot - 1), perf_mode=DR)
        i += 1
    if odd:
        la, ra = leftover
        nc.tensor.matmul(bank, la, ra, start=(i == 0), stop=(i == tot - 1))


def build_module():
    nc = bacc.Bacc(target_bir_lowering=False, trn_type="TRN2")

    ep_r = nc.dram_tensor("ep_r", [P, KC, 2, T], F8, kind="ExternalInput")
    ep_i = nc.dram_tensor("ep_i", [P, KC, 2, T], F8, kind="ExternalInput")
    ep_ni = nc.dram_tensor("ep_ni", [P, KC, 2, T], F8, kind="ExternalInput")
    ep_s = nc.dram_tensor("ep_s", [P, KC, 2, T], F8, kind="ExternalInput")
    ep_d = nc.dram_tensor("ep_d", [P, KC, 2, T], F8, kind="ExternalInput")
    wp_r = nc.dram_tensor("wp_r", [MB, P, KC, 2, P], F8, kind="ExternalInput")
    wp_i = nc.dram_tensor("wp_i", [MB, P, KC, 2, P], F8, kind="ExternalInput")
    wp_s = nc.dram_tensor("wp_s", [MB, P, KC, 2, P], F8, kind="ExternalInput")
    vp_r = nc.dram_tensor("vp_r", [KC, P, 2, D], F8, kind="ExternalInput")
    vp_i = nc.dram_tensor("vp_i", [KC, P, 2, D], F8, kind="ExternalInput")
    vp_s = nc.dram_tensor("vp_s", [KC, P, 2, D], F8, kind="ExternalInput")
    trimask = nc.dram_tensor("trimask", [P, P], F32, kind="ExternalInput")
    rho2 = nc.dram_tensor("rho2", [P, TB], F32, kind="ExternalInput")
    outT_re = nc.dram_tensor("outT_re", [T, D], F32, kind="ExternalOutput")
    outT_im = nc.dram_tensor("outT_im", [T, D], F32, kind="ExternalOutput")

    _n = [0]

    def uid():
        _n[0] += 1
        return _n[0]

    with tile.TileContext(nc) as tc:
        with tc.tile_pool(name="dram", bufs=1, space="DRAM") as dram, \
             tc.tile_pool(name="erp", bufs=1) as erp, \
             tc.tile_pool(name="cst", bufs=1) as cst:
            q = dram.tile([MB, NSP, P, 6, SPAN], F8, tag="q")
            pt = dram.tile([TB, P, 6, D], F8, tag="pt")
            s = dram.tile([TB, TB, P, 6, P], F8, tag="s")

            er = erp.tile([P, KC, 2, T], F8, tag="er")
            mask_sb = cst.tile([P, P], F32, tag="mask")
            rho_sb = cst.tile([P, TB], F32, tag="rho")

            def site_evac(pp, width, c, pk_h_ap, pk_l_ap, ev_pool, rc_pool,
                          masks=None):
                """Evacuate one complex site.

                pp: psum tile [P, 3, width] with banks (M2, M1, M3).
                pk_h_ap/pk_l_ap: output APs for h/l fp8 splits of
                (re, im, sum), or None to skip splits (phase C).
                Returns ev tile [P, 4, width] = (re, im, sum, tt).
                """
                n = uid()
                rc = rc_pool.tile([P, 3, width], F32, tag="rc", name=f"rc{n}")
                ev = ev_pool.tile([P, 4, width], F32, tag="ev", name=f"ev{n}")
                nc.scalar.activation(rc[:], pp[:], COPY)
                nc.vector.tensor_sub(ev[:, 0::3], rc[:, 1:3], rc[:, 0:2])
                nc.gpsimd.tensor_sub(ev[:, 1], ev[:, 3], rc[:, 0])
                if masks is not None:
                    for dsl in masks:
                        nc.vector.tensor_mul(ev[:, 0, dsl], ev[:, 0, dsl],
                                             mask_sb[:])
                        nc.vector.tensor_mul(ev[:, 1, dsl], ev[:, 1, dsl],
                                             mask_sb[:])
                if pk_h_ap is None:
                    return ev
                nc.gpsimd.tensor_add(ev[:, 2], ev[:, 0], ev[:, 1])
                pieces = pk_h_ap if isinstance(pk_h_ap, list) \
                    else [(pk_h_ap, pk_l_ap, slice(None))]
                for h_ap, l_ap, csl in pieces:
                    nc.scalar.activation(h_ap, ev[:, 0:3, csl], COPY, scale=c)
                    nc.vector.scalar_tensor_tensor(
                        out=l_ap, in0=ev[:, 0:3, csl], scalar=c, in1=h_ap,
                        op0=AluOpType.mult, op1=AluOpType.subtract)
                return ev

            # =============== Phases A1 + A2 (merged psum scope) ===========
            qsbp_cm = tc.tile_pool(name="qsbp", bufs=2)
            qsbp = qsbp_cm.__enter__()
            bd01_cm = tc.tile_pool(name="bd01", bufs=1)
            bd01 = bd01_cm.__enter__()
            nei01 = bd01.tile([P, KC, 2, T // 2], F8, tag="nei01")
            ed01 = bd01.tile([P, KC, 2, T // 2], F8, tag="ed01")
            qsb_tiles = {}

            def load_qsb(sp):
                t = qsbp.tile([P, MB, 6, SPAN], F8, tag="qsb",
                              name=f"qsb{sp}")
                nc.sync.dma_start(
                    t[:], q[:, sp].rearrange("m p v t -> p m v t"))
                qsb_tiles[sp] = t

            with tc.tile_pool(name="eip", bufs=1) as eip, \
                 tc.tile_pool(name="esp", bufs=1) as esp:
                # ei/es are rolling 2-panel windows (A2+A1 consume jp-wise)
                eiw = [eip.tile([P, KC, 2, A1W], F8, tag=f"eiw{h}",
                                name=f"eiw{h}") for h in range(2)]
                esw = [esp.tile([P, KC, 2, A1W], F8, tag=f"esw{h}",
                                name=f"esw{h}") for h in range(2)]

                with tc.tile_pool(name="psA", bufs=2, space="PSUM") as psA, \
                     tc.tile_pool(name="rcA", bufs=2) as rcA, \
                     tc.tile_pool(name="evA", bufs=2) as evA, \
                     tc.tile_pool(name="pkQ", bufs=3) as pkQ, \
                     tc.tile_pool(name="pkP", bufs=2) as pkP, \
                     tc.tile_pool(name="wroll", bufs=3) as wrollp, \
                     tc.tile_pool(name="vres", bufs=1) as vres:
                    vr = vres.tile([P, KC, 2, D], F8, tag="vr")
                    vi = vres.tile([P, KC, 2, D], F8, tag="vi")
                    vs = vres.tile([P, KC, 2, D], F8, tag="vs")

                    w_tiles = {}

                    def load_w(key, m):
                        n = uid()
                        wrm = wrollp.tile([P, KC, 2, P], F8, tag="wr",
                                          name=f"wr{n}")
                        wim = wrollp.tile([P, KC, 2, P], F8, tag="wi",
                                          name=f"wi{n}")
                        wsm = wrollp.tile([P, KC, 2, P], F8, tag="ws",
                                          name=f"ws{n}")
                        nc.sync.dma_start(wrm[:], wp_r[m])
                        nc.sync.dma_start(wim[:], wp_i[m])
                        nc.sync.dma_start(wsm[:], wp_s[m])
                        w_tiles[key] = (wrm, wim, wsm)

                    def ewin_load(jp):
                        js = bass.ds(jp * A1W, A1W)
                        h = jp % 2
                        nc.sync.dma_start(eiw[h][:], ep_i[:, :, :, js])
                        nc.sync.dma_start(er[:, :, :, js], ep_r[:, :, :, js])
                        nc.sync.dma_start(esw[h][:], ep_s[:, :, :, js])

                    js0 = bass.ds(0, A1W)
                    nc.sync.dma_start(eiw[0][:], ep_i[:, :, :, js0])
                    nc.sync.dma_start(
                        vi[:], vp_i[:].rearrange("k p s m -> p k s m"))
                    nc.sync.dma_start(er[:, :, :, js0], ep_r[:, :, :, js0])
                    nc.sync.dma_start(
                        vr[:], vp_r[:].rearrange("k p s m -> p k s m"))
                    nc.sync.dma_start(esw[0][:], ep_s[:, :, :, js0])
                    nc.sync.dma_start(
                        vs[:], vp_s[:].rearrange("k p s m -> p k s m"))
                    vd = bass.ds(0, D)

                    def a2_site(tb, ei_t, es_t):
                        tbs = bass.ts(tb, P)
                        lsl = bass.ds((tb % 4) * P, P)
                        n = uid()
                        pp = psA.tile([P, 3, D], F32, tag="pp", name=f"pp{n}")
                        dr_product(nc, pp[:, 0], pack_h0(ei_t, lsl),
                                   pack_h1(vi, vd), KC)
                        dr_product(nc, pp[:, 1], pack_h0(er, tbs),
                                   pack_h1(vr, vd), KC)
                        dr_product(nc, pp[:, 2], pack_h0(es_t, lsl),
                                   pack_h1(vs, vd), KC)
                        ppk = pkP.tile([P, 6, D], F8, tag="pk",
                                       name=f"ppk{n}")
                        site_evac(pp, D, CQ, ppk[:, 1::2], ppk[:, 0::2],
                                  evA, rcA)
                        nc.sync.dma_start(pt[tb], ppk[:])

                    def a1_site(jp, m, ei_t, es_t):
                        js = bass.ds(jp * A1W, A1W)
                        fw = bass.ds(0, A1W)
                        fp128 = bass.ds(0, P)
                        wrm, wim, wsm = w_tiles.pop((jp, m))
                        n = uid()
                        pp = psA.tile([P, 3, A1W], F32, tag="pp",
                                      name=f"pp{n}")
                        dr_product(nc, pp[:, 0], pack_h1(wim, fp128),
                                   pack_h0(ei_t, fw), KC)
                        dr_product(nc, pp[:, 1], pack_h1(wrm, fp128),
                                   pack_h0(er, js), KC)
                        dr_product(nc, pp[:, 2], pack_h1(wsm, fp128),
                                   pack_h0(es_t, fw), KC)
                        qpk = pkQ.tile([P, 2, 6, SPAN], F8, tag="qpk",
                                       name=f"qpk{n}")
                        pieces = [(qpk[:, h, 1::2, :], qpk[:, h, 0::2, :],
                                   slice(h * SPAN, (h + 1) * SPAN))
                                  for h in range(2)]
                        site_evac(pp, A1W, CQ, pieces, None, evA, rcA)
                        nc.sync.dma_start(q[m, 2 * jp], qpk[:, 0])
                        nc.sync.dma_start(q[m, 2 * jp + 1], qpk[:, 1])

                    pairs = [(jp, m) for jp in range(NJP) for m in range(MB)]
                    for jp in range(NJP):
                        ei_t, es_t = eiw[jp % 2], esw[jp % 2]
                        a2_site(4 * jp + 0, ei_t, es_t)
                        if jp == 0:
                            load_w((0, 0), 0)
                            load_w((0, 1), 1)
                        a2_site(4 * jp + 1, ei_t, es_t)
                        if jp == 0:
                            nc.sync.dma_start(mask_sb[:], trimask[:])
                            nc.sync.dma_start(rho_sb[:], rho2[:])
                        a2_site(4 * jp + 2, ei_t, es_t)
                        a2_site(4 * jp + 3, ei_t, es_t)
                        if jp + 1 < NJP:
                            ewin_load(jp + 1)
                        if jp in (1, 2):
                            hq = bass.ds((jp - 1) * A1W, A1W)
                            nc.sync.dma_start(nei01[:, :, :, hq],
                                              ep_ni[:, :, :, hq])
                            nc.sync.dma_start(ed01[:, :, :, hq],
                                              ep_d[:, :, :, hq])
                        for m in range(MB):
                            idx = jp * MB + m
                            if idx + 2 < len(pairs):
                                load_w(pairs[idx + 2], pairs[idx + 2][1])
                            a1_site(jp, m, ei_t, es_t)
                            if jp == 0 and m == MB - 1:
                                load_qsb(0)
                                load_qsb(1)

            # =============== Phase B: S = E^H Q (upper tri) ===============
            with tc.tile_pool(name="ptp", bufs=1) as ptpp:
                ptr = ptpp.tile([P, TB, 2, D], F8, tag="ptr")
                pti = ptpp.tile([P, TB, 2, D], F8, tag="pti")

                with tc.tile_pool(name="psB", bufs=3, space="PSUM") as psB, \
                     tc.tile_pool(name="rcB", bufs=3) as rcB, \
                     tc.tile_pool(name="evB", bufs=3) as evB, \
                     tc.tile_pool(name="spkp", bufs=3) as spkp, \
                     tc.tile_pool(name="ptsp", bufs=1) as ptsp:
                  pts = ptsp.tile([P, TB, 2, D], F8, tag="pts")
                  sst_small = {}
                  with tc.tile_pool(name="edp", bufs=1) as edp:
                    nei23 = edp.tile([P, KC, 2, T // 2], F8, tag="nei23")
                    ed23 = edp.tile([P, KC, 2, T // 2], F8, tag="ed23")
                    nc.sync.dma_start(
                        pts[:], pt[:, :, 4:6].rearrange("t p v d -> p t v d"))

                    def b_lhs(t01, t23, ib):
                        if ib < MB:
                            return pack_h0(t01, bass.ts(ib, P))
                        return pack_h0(t23, bass.ts(ib - MB, P))
                    nc.sync.dma_start(
                        ptr[:], pt[:, :, 0:2].rearrange("t p v d -> p t v d"))
                    nc.sync.dma_start(
                        pti[:], pt[:, :, 2:4].rearrange("t p v d -> p t v d"))

                    for sp in range(NSP):
                        if sp + 2 < NSP:
                            load_qsb(sp + 2)
                        if sp < 2:
                            lq = bass.ds(sp * A1W, A1W)
                            gq = bass.ds(T // 2 + sp * A1W, A1W)
                            nc.sync.dma_start(nei23[:, :, :, lq],
                                              ep_ni[:, :, :, gq])
                            nc.sync.dma_start(ed23[:, :, :, lq],
                                              ep_d[:, :, :, gq])
                        if sp == 2:
                            for _jb in range(2):
                                t = spkp.tile([P, 2, 6, P], F8, tag="sst_s",
                                              name=f"sst_s{_jb}")[:, :_jb + 1]
                                nc.sync.dma_start(
                                    t[:], s[:_jb + 1, _jb].rearrange(
                                        "i p v j -> p i v j"))
                                sst_small[_jb] = t
                        qsb = qsb_tiles.pop(sp)

                        def rhs_q(vb):
                            def f(k, kind):
                                if kind == "hh":
                                    return qsb[:, 2 * k:2 * k + 2, vb + 1, :]
                                return qsb[:, k, vb:vb + 2, :]
                            return f

                        for ib in range(2 * sp + 2):
                            ibs = bass.ts(ib, P)
                            top = ib == 2 * sp + 1  # low half would be garbage
                            w = P if top else SPAN

                            def rq(vb, _top=top):
                                base = rhs_q(vb)
                                if not _top:
                                    return base

                                def f(k, kind):
                                    return base(k, kind)[:, :, P:]
                                return f

                            n = uid()
                            pp = psB.tile([P, 3, SPAN], F32, tag="pp",
                                          name=f"pp{n}")[:, :, :w]
                            dr_product(nc, pp[:, 0], b_lhs(nei01, nei23, ib),
                                       rq(2), KC)
                            dr_product(nc, pp[:, 1], pack_h0(er, ibs),
                                       rq(0), KC)
                            dr_product(nc, pp[:, 2], b_lhs(ed01, ed23, ib),
                                       rq(4), KC)
                            masks = [bass.ds(0, P)] if (
                                top or ib == 2 * sp) else []
                            spk = spkp.tile([P, 2, 6, P], F8, tag="spk",
                                            name=f"spk{n}")
                            nh = 1 if top else 2
                            pieces = [(spk[:, jh, 0::2, :],
                                       spk[:, jh, 1::2, :],
                                       slice(jh * P, (jh + 1) * P))
                                      for jh in range(nh)]
                            site_evac(pp, w, CS, pieces, None,
                                      evB, rcB, masks=masks)
                            for jh in range(nh):
                                jb = 2 * sp + (1 if top else jh)
                                if ib <= jb:
                                    nc.sync.dma_start(s[ib, jb],
                                                      spk[:, jh])

                  # ======== Phase C (shares psB/rcB/evB pools) ========
                  with tc.tile_pool(name="sstp", bufs=2) as sstp, \
                       tc.tile_pool(name="out4", bufs=3) as out4:
                    sst_tiles = {}

                    def load_sst(jb):
                        t = sstp.tile([P, TB, 6, P], F8, tag="sst",
                                      name=f"sst{jb}")[:, :jb + 1]
                        nc.sync.dma_start(
                            t[:], s[:jb + 1, jb].rearrange(
                                "i p v j -> p i v j"))
                        sst_tiles[jb] = t

                    sst_tiles.update(sst_small)
                    load_sst(2)
                    order = list(range(1, TB)) + [0]
                    for oi, jb in enumerate(order):
                        jbs = bass.ts(jb, P)
                        nk = jb + 1
                        nxt = order[oi + 1] if oi + 1 < TB else None
                        if nxt is not None and nxt >= 2 and nxt + 1 <= TB:
                            pass
                        if jb + 2 <= TB - 1 + 1 and 2 <= jb + 1 < TB:
                            load_sst(jb + 1)
                        sst = sst_tiles.pop(jb)

                        def lhs_s(vb):
                            def f(k, kind):
                                if kind == "hh":
                                    return sst[:, 2 * k:2 * k + 2, vb, :]
                                return sst[:, k, vb:vb + 2, :]
                            return f

                        kl = nk - 1
                        oo = out4.tile([P, 2, D], F32, tag="oo",
                                       name=f"oo{jb}")
                        for ch in range(2):
                            cds = bass.ds(ch * SPAN, SPAN)
                            n = uid()
                            pp = psB.tile([P, 3, SPAN], F32, tag="pp",
                                          name=f"pp{n}")
                            dr_product(nc, pp[:, 0], lhs_s(2),
                                       pack_h1(pti, cds), nk,
                                       leftover=(sst[:, kl, 2, :],
                                                 pti[:, kl, 1, cds]))
                            dr_product(nc, pp[:, 1], lhs_s(0),
                                       pack_h1(ptr, cds), nk,
                                       leftover=(sst[:, kl, 0, :],
                                                 ptr[:, kl, 1, cds]))
                            dr_product(nc, pp[:, 2], lhs_s(4),
                                       pack_h1(pts, cds), nk,
                                       leftover=(sst[:, kl, 4, :],
                                                 pts[:, kl, 1, cds]))
                            ev = site_evac(pp, SPAN, None, None, None,
                                           evB, rcB)
                            nc.scalar.activation(
                                oo[:, :, cds], ev[:, 0:2], COPY,
                                scale=rho_sb[:, jb:jb + 1])
                        nc.sync.dma_start(outT_re[jbs, :], oo[:, 0])
                        nc.sync.dma_start(outT_im[jbs, :], oo[:, 1])
            bd01_cm.__exit__(None, None, None)
            qsbp_cm.__exit__(None, None, None)

    nc.compile()
    return nc


_NC_CACHE = None


def _get_module():
    global _NC_CACHE
    if _NC_CACHE is None:
        _NC_CACHE = build_module()
    return _NC_CACHE


def _split(x):
    h = x.astype(E4NP)
    l = (x - h.astype(np.float32)).astype(E4NP)
    return h, l


def _pack(x, hfirst):
    """x [D2, F] f32 -> fp8 pack: [P, KC, 2, F] (E, h-first) or
    [KC, P, 2, F] (weights, l-first)."""
    h, l = _split(x)
    F = x.shape[1]
    if hfirst:
        out = np.empty((P, KC, 2, F), E4NP)
        out[:, :, 0] = h.reshape(KC, P, F).transpose(1, 0, 2)
        out[:, :, 1] = l.reshape(KC, P, F).transpose(1, 0, 2)
    else:
        out = np.empty((KC, P, 2, F), E4NP)
        out[:, :, 1] = h.reshape(KC, P, F)
        out[:, :, 0] = l.reshape(KC, P, F)
    return out


def _pack_w(w):
    """w [D2, D2] (c, m) f32 -> [MB, P, KC, 2(l,h), P] fp8 pack."""
    h, l = _split(w)
    out = np.empty((MB, P, KC, 2, P), E4NP)
    out[:, :, :, 1] = h.reshape(KC, P, MB, P).transpose(2, 1, 0, 3)
    out[:, :, :, 0] = l.reshape(KC, P, MB, P).transpose(2, 1, 0, 3)
    return out


def prep_shared(WKQ_re, WKQ_im, WPV_re, WPV_im):
    wr = np.ascontiguousarray(WKQ_re.T) * 256.0
    wi = np.ascontiguousarray(WKQ_im.T) * 256.0
    vr = np.ascontiguousarray(WPV_re.T) * 256.0
    vi = np.ascontiguousarray(WPV_im.T) * 256.0
    shared = {
        "wp_r": _pack_w(wr), "wp_i": _pack_w(wi),
        "wp_s": _pack_w(wr + wi),
        "vp_r": _pack(vr, False), "vp_i": _pack(vi, False),
        "vp_s": _pack(vr + vi, False),
        "trimask": np.triu(np.ones((P, P), np.float32)),
    }
    j = np.arange(T, dtype=np.float32)
    rho = 2.0 / np.maximum(j, 1.0)
    shared["rho2"] = np.ascontiguousarray(rho.reshape(TB, P).T)
    return shared


def kernel(E_re, E_im, WKQ_re, WKQ_im, WPV_re, WPV_im):
    E_re = np.asarray(E_re, dtype=np.float32)
    E_im = np.asarray(E_im, dtype=np.float32)
    shared = prep_shared(np.asarray(WKQ_re, np.float32),
                         np.asarray(WKQ_im, np.float32),
                         np.asarray(WPV_re, np.float32),
                         np.asarray(WPV_im, np.float32))
    in_maps = []
    for b in range(B):
        er = E_re[b] * 4.0
        ei = E_im[b] * 4.0
        m = dict(shared)
        m["ep_r"] = _pack(er, True)
        m["ep_i"] = _pack(ei, True)
        m["ep_ni"] = _pack(-ei, True)
        m["ep_s"] = _pack(er + ei, True)
        m["ep_d"] = _pack(er - ei, True)
        in_maps.append(m)

    nc = _get_module()
    res = run_bass_kernel_spmd(nc, in_maps, core_ids=list(range(B)))

    out = np.empty((B, D, T - 2), dtype=np.complex64)
    for b in range(B):
        r = res.results[b]["outT_re"]  # [T, D]
        i = res.results[b]["outT_im"]
        full = (r + 1j * i.astype(np.complex64)).T  # [D, T]
        out[b] = full[:, 1:T - 1]
    return out


# revision 45
# speedup vs baseline: 1.1002x; 1.0312x over previous
"""Trainium2 Bass kernel for nn_AutoregressiveLSA — fp8 DoubleRow version.

Math (complex, per batch b, one NeuronCore per batch element):
    Q  = WKQ @ E                       [2d, T]
    S  = E^H @ Q, keep i <= j          [T, T]
    outT[j] = sum_{i<=j} S[i,j] PT[i] * 2/max(j,1),  PT = (WPV @ E)^T

All matmuls run as fp8e4 (e4m3) in DoubleRow perf mode: one PE
instruction contracts TWO 128-chunks at 0.5 cycles/output-column (4x
the fp32r MAC rate).  Precision comes from a hi/lo split of every
operand (x ~ x_h + x_l, both e4m3; x_l*y_l dropped): per 128-chunk each
real product needs 3 fp8 pairings = 1.5 DR instructions, so a complex
Karatsuba product costs 2.25 free-columns/chunk vs 3.0 for fp32r.
Measured end-to-end rel err ~3e-3 (gate 2e-2).

Scale chain (powers of 2, folded into casts / final rho):
    E*4, WKQ^T*256, WPV^T*256 quantized on host.
    A1 psum = 1024*Q,  split scale 2^-7  -> Q'' = 8Q
    A2 psum = 1024*PT, split scale 2^-7  -> PT'' = 8PT
    B  psum = 32*S,    split scale 2^-9  -> S'' = S/16
    C  psum = S*PT/2,  rho2 = 2/max(j,1) applied via Act scale.

Engine constraints honored (probed on real TRN2): vector ops may read
at most ONE psum operand; Pool (gpsimd) runs SBUF-only tensor_tensor
(no psum, no scalar_tensor_tensor); Act does scaled copies (fp8 out ok).
Evacuation is fused into wide ops: psum banks ordered (M2, M1, M3) so
one 3W psum->sbuf copy + one dual-sub [re,tt] + pool im/sum + ONE 3W
Act h-cast + ONE 3W DVE stt l-split handle a whole complex site.
Phase B uses a host-negated E_im pack (nei) so its conjugated
recombination has the same (M1-M2', M3-M1-M2') form as the others.
"""

import numpy as np
import ml_dtypes

import concourse.bass as bass
import concourse.mybir as mybir
import concourse.tile as tile
from concourse import bacc
from concourse.bass_utils import run_bass_kernel_spmd
from concourse.alu_op_type import AluOpType

F32 = mybir.dt.float32
F8 = mybir.dt.float8e4
E4NP = ml_dtypes.float8_e4m3
DR = mybir.MatmulPerfMode.DoubleRow
COPY = mybir.ActivationFunctionType.Copy

B = 8
D2 = 1024
T = 2048
D = 512
P = 128
KC = D2 // P
MB = D2 // P
TB = T // P
A1W = 512
NJP = T // A1W
SPAN = 256
NSP = T // SPAN

CQ = float(2.0 ** -7)
CS = float(2.0 ** -9)


def pack_h0(t, fsl):
    """Slicer for h-first packs [P, K, 2(h,l), F] (E/S side)."""
    def f(k, kind):
        if kind == "hh":
            return t[:, 2 * k:2 * k + 2, 0, fsl]
        return t[:, k, :, fsl]
    return f


def pack_h1(t, fsl):
    """Slicer for l-first packs [P, K, 2(l,h), F] (W/Q/PT side)."""
    def f(k, kind):
        if kind == "hh":
            return t[:, 2 * k:2 * k + 2, 1, fsl]
        return t[:, k, :, fsl]
    return f


def dr_product(nc, bank, lhs, rhs, nk, leftover=None):
    nhh = nk // 2
    odd = nk % 2
    tot = nhh + nk + (1 if odd else 0)
    i = 0
    for kp in range(nhh):
        nc.tensor.matmul(bank, lhs(kp, "hh"), rhs(kp, "hh"),
                         start=(i == 0), stop=(i == tot - 1), perf_mode=DR)
        i += 1
    for k in range(nk):
        nc.tensor.matmul(bank, lhs(k, "x"), rhs(k, "x"),
                         start=(i == 0), stop=(i == tot - 1), perf_mode=DR)
        i += 1
    if odd:
        la, ra = leftover
        nc.tensor.matmul(bank, la, ra, start=(i == 0), stop=(i == tot - 1))


def build_module():
    nc = bacc.Bacc(target_bir_lowering=False, trn_type="TRN2")

    ep_r = nc.dram_tensor("ep_r", [P, KC, 2, T], F8, kind="ExternalInput")
    ep_i = nc.dram_tensor("ep_i", [P, KC, 2, T], F8, kind="ExternalInput")
    ep_ni = nc.dram_tensor("ep_ni", [P, KC, 2, T], F8, kind="ExternalInput")
    ep_s = nc.dram_tensor("ep_s", [P, KC, 2, T], F8, kind="ExternalInput")
    ep_d = nc.dram_tensor("ep_d", [P, KC, 2, T], F8, kind="ExternalInput")
    wp_r = nc.dram_tensor("wp_r", [MB, P, KC, 2, P], F8, kind="ExternalInput")
    wp_i = nc.dram_tensor("wp_i", [MB, P, KC, 2, P], F8, kind="ExternalInput")
    wp_s = nc.dram_tensor("wp_s", [MB, P, KC, 2, P], F8, kind="ExternalInput")
    vp_r = nc.dram_tensor("vp_r", [KC, P, 2, D], F8, kind="ExternalInput")
    vp_i = nc.dram_tensor("vp_i", [KC, P, 2, D], F8, kind="ExternalInput")
    vp_s = nc.dram_tensor("vp_s", [KC, P, 2, D], F8, kind="ExternalInput")
    trimask = nc.dram_tensor("trimask", [P, P], F32, kind="ExternalInput")
    rho2 = nc.dram_tensor("rho2", [P, TB], F32, kind="ExternalInput")
    outT_re = nc.dram_tensor("outT_re", [T, D], F32, kind="ExternalOutput")
    outT_im = nc.dram_tensor("outT_im", [T, D], F32, kind="ExternalOutput")

    _n = [0]

    def uid():
        _n[0] += 1
        return _n[0]

    with tile.TileContext(nc) as tc:
        with tc.tile_pool(name="dram", bufs=1, space="DRAM") as dram, \
             tc.tile_pool(name="erp", bufs=1) as erp, \
             tc.tile_pool(name="cst", bufs=1) as cst:
            q = dram.tile([MB, NSP, P, 6, SPAN], F8, tag="q")
            pt = dram.tile([TB, P, 6, D], F8, tag="pt")
            s = dram.tile([TB, TB, P, 6, P], F8, tag="s")

            er = erp.tile([P, KC, 2, T], F8, tag="er")
            mask_sb = cst.tile([P, P], F32, tag="mask")
            rho_sb = cst.tile([P, TB], F32, tag="rho")

            def site_evac(pp, width, c, pk_h_ap, pk_l_ap, ev_pool, rc_pool,
                          masks=None):
                """Evacuate one complex site.

                pp: psum tile [P, 3, width] with banks (M2, M1, M3).
                pk_h_ap/pk_l_ap: output APs for h/l fp8 splits of
                (re, im, sum), or None to skip splits (phase C).
                Returns ev tile [P, 4, width] = (re, im, sum, tt).
                """
                n = uid()
                rc = rc_pool.tile([P, 3, width], F32, tag="rc", name=f"rc{n}")
                ev = ev_pool.tile([P, 4, width], F32, tag="ev", name=f"ev{n}")
                nc.scalar.activation(rc[:], pp[:], COPY)
                nc.vector.tensor_sub(ev[:, 0::3], rc[:, 1:3], rc[:, 0:2])
                nc.gpsimd.tensor_sub(ev[:, 1], ev[:, 3], rc[:, 0])
                if masks is not None:
                    for dsl in masks:
                        nc.vector.tensor_mul(ev[:, 0, dsl], ev[:, 0, dsl],
                                             mask_sb[:])
                        nc.vector.tensor_mul(ev[:, 1, dsl], ev[:, 1, dsl],
                                             mask_sb[:])
                if pk_h_ap is None:
                    return ev
                nc.gpsimd.tensor_add(ev[:, 2], ev[:, 0], ev[:, 1])
                pieces = pk_h_ap if isinstance(pk_h_ap, list) \
                    else [(pk_h_ap, pk_l_ap, slice(None))]
                for h_ap, l_ap, csl in pieces:
                    nc.scalar.activation(h_ap, ev[:, 0:3, csl], COPY, scale=c)
                    nc.vector.scalar_tensor_tensor(
                        out=l_ap, in0=ev[:, 0:3, csl], scalar=c, in1=h_ap,
                        op0=AluOpType.mult, op1=AluOpType.subtract)
                return ev

            # =============== Phases A1 + A2 (merged psum scope) ===========
            qsbp_cm = tc.tile_pool(name="qsbp", bufs=2)
            qsbp = qsbp_cm.__enter__()
            bd01_cm = tc.tile_pool(name="bd01", bufs=1)
            bd01 = bd01_cm.__enter__()
            nei01 = bd01.tile([P, KC, 2, T // 2], F8, tag="nei01")
            ed01 = bd01.tile([P, KC, 2, T // 2], F8, tag="ed01")
            qsb_tiles = {}

            def load_qsb(sp):
                t = qsbp.tile([P, MB, 6, SPAN], F8, tag="qsb",
                              name=f"qsb{sp}")
                nc.sync.dma_start(
                    t[:], q[:, sp].rearrange("m p v t -> p m v t"))
                qsb_tiles[sp] = t

            with tc.tile_pool(name="eip", bufs=1) as eip, \
                 tc.tile_pool(name="esp", bufs=1) as esp:
                # ei/es are rolling 2-panel windows (A2+A1 consume jp-wise)
                eiw = [eip.tile([P, KC, 2, A1W], F8, tag=f"eiw{h}",
                                name=f"eiw{h}") for h in range(2)]
                esw = [esp.tile([P, KC, 2, A1W], F8, tag=f"esw{h}",
                                name=f"esw{h}") for h in range(2)]

                with tc.tile_pool(name="psA", bufs=2, space="PSUM") as psA, \
                     tc.tile_pool(name="rcA", bufs=2) as rcA, \
                     tc.tile_pool(name="evA", bufs=2) as evA, \
                     tc.tile_pool(name="pkQ", bufs=3) as pkQ, \
                     tc.tile_pool(name="pkP", bufs=2) as pkP, \
                     tc.tile_pool(name="wroll", bufs=3) as wrollp, \
                     tc.tile_pool(name="vres", bufs=1) as vres:
                    vr = vres.tile([P, KC, 2, D], F8, tag="vr")
                    vi = vres.tile([P, KC, 2, D], F8, tag="vi")
                    vs = vres.tile([P, KC, 2, D], F8, tag="vs")

                    w_tiles = {}

                    def load_w(key, m):
                        n = uid()
                        wrm = wrollp.tile([P, KC, 2, P], F8, tag="wr",
                                          name=f"wr{n}")
                        wim = wrollp.tile([P, KC, 2, P], F8, tag="wi",
                                          name=f"wi{n}")
                        wsm = wrollp.tile([P, KC, 2, P], F8, tag="ws",
                                          name=f"ws{n}")
                        nc.sync.dma_start(wrm[:], wp_r[m])
                        nc.sync.dma_start(wim[:], wp_i[m])
                        nc.sync.dma_start(wsm[:], wp_s[m])
                        w_tiles[key] = (wrm, wim, wsm)

                    def ewin_load(jp):
                        js = bass.ds(jp * A1W, A1W)
                        h = jp % 2
                        nc.sync.dma_start(eiw[h][:], ep_i[:, :, :, js])
                        nc.sync.dma_start(er[:, :, :, js], ep_r[:, :, :, js])
                        nc.sync.dma_start(esw[h][:], ep_s[:, :, :, js])

                    js0 = bass.ds(0, A1W)
                    nc.sync.dma_start(eiw[0][:], ep_i[:, :, :, js0])
                    nc.sync.dma_start(
                        vi[:], vp_i[:].rearrange("k p s m -> p k s m"))
                    nc.sync.dma_start(er[:, :, :, js0], ep_r[:, :, :, js0])
                    nc.sync.dma_start(
                        vr[:], vp_r[:].rearrange("k p s m -> p k s m"))
                    nc.sync.dma_start(esw[0][:], ep_s[:, :, :, js0])
                    nc.sync.dma_start(
                        vs[:], vp_s[:].rearrange("k p s m -> p k s m"))
                    vd = bass.ds(0, D)

                    def a2_site(tb, ei_t, es_t):
                        tbs = bass.ts(tb, P)
                        lsl = bass.ds((tb % 4) * P, P)
                        n = uid()
                        pp = psA.tile([P, 3, D], F32, tag="pp", name=f"pp{n}")
                        dr_product(nc, pp[:, 0], pack_h0(ei_t, lsl),
                                   pack_h1(vi, vd), KC)
                        dr_product(nc, pp[:, 1], pack_h0(er, tbs),
                                   pack_h1(vr, vd), KC)
                        dr_product(nc, pp[:, 2], pack_h0(es_t, lsl),
                                   pack_h1(vs, vd), KC)
                        ppk = pkP.tile([P, 6, D], F8, tag="pk",
                                       name=f"ppk{n}")
                        site_evac(pp, D, CQ, ppk[:, 1::2], ppk[:, 0::2],
                                  evA, rcA)
                        nc.sync.dma_start(pt[tb], ppk[:])

                    def a1_site(jp, m, ei_t, es_t, pop=True, key=None):
                        js = bass.ds(jp * A1W, A1W)
                        fw = bass.ds(0, A1W)
                        fp128 = bass.ds(0, P)
                        key = key if key is not None else (jp // 2, m)
                        if pop:
                            wrm, wim, wsm = w_tiles.pop(key)
                        else:
                            wrm, wim, wsm = w_tiles[key]
                        n = uid()
                        pp = psA.tile([P, 3, A1W], F32, tag="pp",
                                      name=f"pp{n}")
                        dr_product(nc, pp[:, 0], pack_h1(wim, fp128),
                                   pack_h0(ei_t, fw), KC)
                        dr_product(nc, pp[:, 1], pack_h1(wrm, fp128),
                                   pack_h0(er, js), KC)
                        dr_product(nc, pp[:, 2], pack_h1(wsm, fp128),
                                   pack_h0(es_t, fw), KC)
                        qpk = pkQ.tile([P, 2, 6, SPAN], F8, tag="qpk",
                                       name=f"qpk{n}")
                        pieces = [(qpk[:, h, 1::2, :], qpk[:, h, 0::2, :],
                                   slice(h * SPAN, (h + 1) * SPAN))
                                  for h in range(2)]
                        site_evac(pp, A1W, CQ, pieces, None, evA, rcA)
                        nc.sync.dma_start(q[m, 2 * jp], qpk[:, 0])
                        nc.sync.dma_start(q[m, 2 * jp + 1], qpk[:, 1])

                    pairs = [(jp, m) for jp in range(NJP) for m in range(MB)]
                    for half in range(2):
                        jpa, jpb = 2 * half, 2 * half + 1
                        if half == 1:
                            ewin_load(2)
                            ewin_load(3)
                        a2_site(8 * half + 0, eiw[0], esw[0])
                        if half == 0:
                            load_w((0, 0), 0)
                            load_w((0, 1), 1)
                        a2_site(8 * half + 1, eiw[0], esw[0])
                        if half == 0:
                            nc.sync.dma_start(mask_sb[:], trimask[:])
                            nc.sync.dma_start(rho_sb[:], rho2[:])
                            ewin_load(1)
                        a2_site(8 * half + 2, eiw[0], esw[0])
                        a2_site(8 * half + 3, eiw[0], esw[0])
                        if half == 1:
                            hq = bass.ds(0, A1W)
                            nc.sync.dma_start(nei01[:, :, :, hq],
                                              ep_ni[:, :, :, hq])
                            nc.sync.dma_start(ed01[:, :, :, hq],
                                              ep_d[:, :, :, hq])
                        a2_site(8 * half + 4, eiw[1], esw[1])
                        a2_site(8 * half + 5, eiw[1], esw[1])
                        if half == 1:
                            hq = bass.ds(A1W, A1W)
                            nc.sync.dma_start(nei01[:, :, :, hq],
                                              ep_ni[:, :, :, hq])
                            nc.sync.dma_start(ed01[:, :, :, hq],
                                              ep_d[:, :, :, hq])
                        a2_site(8 * half + 6, eiw[1], esw[1])
                        a2_site(8 * half + 7, eiw[1], esw[1])
                        for m in range(MB):
                            if m + 2 < MB:
                                load_w((half, m + 2), m + 2)
                            elif half == 0:
                                load_w((1, m + 2 - MB), m + 2 - MB)
                            a1_site(jpa, m, eiw[0], esw[0], pop=False)
                            a1_site(jpb, m, eiw[1], esw[1], pop=True,
                                    key=(half, m))
                            if half == 0 and m == MB - 1:
                                load_qsb(0)
                                load_qsb(1)

            # =============== Phase B: S = E^H Q (upper tri) ===============
            with tc.tile_pool(name="ptp", bufs=1) as ptpp:
                ptr = ptpp.tile([P, TB, 2, D], F8, tag="ptr")
                pti = ptpp.tile([P, TB, 2, D], F8, tag="pti")

                with tc.tile_pool(name="psB", bufs=3, space="PSUM") as psB, \
                     tc.tile_pool(name="rcB", bufs=3) as rcB, \
                     tc.tile_pool(name="evB", bufs=3) as evB, \
                     tc.tile_pool(name="spkp", bufs=3) as spkp, \
                     tc.tile_pool(name="ptsp", bufs=1) as ptsp:
                  pts = ptsp.tile([P, TB, 2, D], F8, tag="pts")
                  sst_small = {}
                  with tc.tile_pool(name="edp", bufs=1) as edp:
                    nei23 = edp.tile([P, KC, 2, T // 2], F8, tag="nei23")
                    ed23 = edp.tile([P, KC, 2, T // 2], F8, tag="ed23")
                    nc.sync.dma_start(
                        pts[:], pt[:, :, 4:6].rearrange("t p v d -> p t v d"))

                    def b_lhs(t01, t23, ib):
                        if ib < MB:
                            return pack_h0(t01, bass.ts(ib, P))
                        return pack_h0(t23, bass.ts(ib - MB, P))
                    nc.sync.dma_start(
                        ptr[:], pt[:, :, 0:2].rearrange("t p v d -> p t v d"))
                    nc.sync.dma_start(
                        pti[:], pt[:, :, 2:4].rearrange("t p v d -> p t v d"))

                    for sp in range(NSP):
                        if sp + 2 < NSP:
                            load_qsb(sp + 2)
                        if sp < 2:
                            lq = bass.ds(sp * A1W, A1W)
                            gq = bass.ds(T // 2 + sp * A1W, A1W)
                            nc.sync.dma_start(nei23[:, :, :, lq],
                                              ep_ni[:, :, :, gq])
                            nc.sync.dma_start(ed23[:, :, :, lq],
                                              ep_d[:, :, :, gq])
                        if sp == 2:
                            for _jb in range(2):
                                t = spkp.tile([P, 2, 6, P], F8, tag="sst_s",
                                              name=f"sst_s{_jb}")[:, :_jb + 1]
                                nc.sync.dma_start(
                                    t[:], s[:_jb + 1, _jb].rearrange(
                                        "i p v j -> p i v j"))
                                sst_small[_jb] = t
                        qsb = qsb_tiles.pop(sp)

                        def rhs_q(vb):
                            def f(k, kind):
                                if kind == "hh":
                                    return qsb[:, 2 * k:2 * k + 2, vb + 1, :]
                                return qsb[:, k, vb:vb + 2, :]
                            return f

                        for ib in range(2 * sp + 2):
                            ibs = bass.ts(ib, P)
                            top = ib == 2 * sp + 1  # low half would be garbage
                            w = P if top else SPAN

                            def rq(vb, _top=top):
                                base = rhs_q(vb)
                                if not _top:
                                    return base

                                def f(k, kind):
                                    return base(k, kind)[:, :, P:]
                                return f

                            n = uid()
                            pp = psB.tile([P, 3, SPAN], F32, tag="pp",
                                          name=f"pp{n}")[:, :, :w]
                            dr_product(nc, pp[:, 0], b_lhs(nei01, nei23, ib),
                                       rq(2), KC)
                            dr_product(nc, pp[:, 1], pack_h0(er, ibs),
                                       rq(0), KC)
                            dr_product(nc, pp[:, 2], b_lhs(ed01, ed23, ib),
                                       rq(4), KC)
                            masks = [bass.ds(0, P)] if (
                                top or ib == 2 * sp) else []
                            spk = spkp.tile([P, 2, 6, P], F8, tag="spk",
                                            name=f"spk{n}")
                            nh = 1 if top else 2
                            pieces = [(spk[:, jh, 0::2, :],
                                       spk[:, jh, 1::2, :],
                                       slice(jh * P, (jh + 1) * P))
                                      for jh in range(nh)]
                            site_evac(pp, w, CS, pieces, None,
                                      evB, rcB, masks=masks)
                            for jh in range(nh):
                                jb = 2 * sp + (1 if top else jh)
                                if ib <= jb:
                                    nc.sync.dma_start(s[ib, jb],
                                                      spk[:, jh])

                  # ======== Phase C (shares psB/rcB/evB pools) ========
                  with tc.tile_pool(name="sstp", bufs=2) as sstp, \
                       tc.tile_pool(name="out4", bufs=3) as out4:
                    sst_tiles = {}

                    def load_sst(jb):
                        t = sstp.tile([P, TB, 6, P], F8, tag="sst",
                                      name=f"sst{jb}")[:, :jb + 1]
                        nc.sync.dma_start(
                            t[:], s[:jb + 1, jb].rearrange(
                                "i p v j -> p i v j"))
                        sst_tiles[jb] = t

                    sst_tiles.update(sst_small)
                    load_sst(2)
                    order = list(range(1, TB)) + [0]
                    for oi, jb in enumerate(order):
                        jbs = bass.ts(jb, P)
                        nk = jb + 1
                        nxt = order[oi + 1] if oi + 1 < TB else None
                        if nxt is not None and nxt >= 2 and nxt + 1 <= TB:
                            pass
                        if jb + 2 <= TB - 1 + 1 and 2 <= jb + 1 < TB:
                            load_sst(jb + 1)
                        sst = sst_tiles.pop(jb)

                        def lhs_s(vb):
                            def f(k, kind):
                                if kind == "hh":
                                    return sst[:, 2 * k:2 * k + 2, vb, :]
                                return sst[:, k, vb:vb + 2, :]
                            return f

                        kl = nk - 1
                        oo = out4.tile([P, 2, D], F32, tag="oo",
                                       name=f"oo{jb}")
                        for ch in range(2):
                            cds = bass.ds(ch * SPAN, SPAN)
                            n = uid()
                            pp = psB.tile([P, 3, SPAN], F32, tag="pp",
                                          name=f"pp{n}")
                            dr_product(nc, pp[:, 0], lhs_s(2),
                                       pack_h1(pti, cds), nk,
                                       leftover=(sst[:, kl, 2, :],
                                                 pti[:, kl, 1, cds]))
                            dr_product(nc, pp[:, 1], lhs_s(0),
                                       pack_h1(ptr, cds), nk,
                                       leftover=(sst[:, kl, 0, :],
                                                 ptr[:, kl, 1, cds]))
                            dr_product(nc, pp[:, 2], lhs_s(4),
                                       pack_h1(pts, cds), nk,
                                       leftover=(sst[:, kl, 4, :],
                                                 pts[:, kl, 1, cds]))
                            ev = site_evac(pp, SPAN, None, None, None,
                                           evB, rcB)
                            nc.scalar.activation(
                                oo[:, :, cds], ev[:, 0:2], COPY,
                                scale=rho_sb[:, jb:jb + 1])
                        nc.sync.dma_start(outT_re[jbs, :], oo[:, 0])
                        nc.sync.dma_start(outT_im[jbs, :], oo[:, 1])
            bd01_cm.__exit__(None, None, None)
            qsbp_cm.__exit__(None, None, None)

    nc.compile()
    return nc


_NC_CACHE = None


def _get_module():
    global _NC_CACHE
    if _NC_CACHE is None:
        _NC_CACHE = build_module()
    return _NC_CACHE


def _split(x):
    h = x.astype(E4NP)
    l = (x - h.astype(np.float32)).astype(E4NP)
    return h, l


def _pack(x, hfirst):
    """x [D2, F] f32 -> fp8 pack: [P, KC, 2, F] (E, h-first) or
    [KC, P, 2, F] (weights, l-first)."""
    h, l = _split(x)
    F = x.shape[1]
    if hfirst:
        out = np.empty((P, KC, 2, F), E4NP)
        out[:, :, 0] = h.reshape(KC, P, F).transpose(1, 0, 2)
        out[:, :, 1] = l.reshape(KC, P, F).transpose(1, 0, 2)
    else:
        out = np.empty((KC, P, 2, F), E4NP)
        out[:, :, 1] = h.reshape(KC, P, F)
        out[:, :, 0] = l.reshape(KC, P, F)
    return out


def _pack_w(w):
    """w [D2, D2] (c, m) f32 -> [MB, P, KC, 2(l,h), P] fp8 pack."""
    h, l = _split(w)
    out = np.empty((MB, P, KC, 2, P), E4NP)
    out[:, :, :, 1] = h.reshape(KC, P, MB, P).transpose(2, 1, 0, 3)
    out[:, :, :, 0] = l.reshape(KC, P, MB, P).transpose(2, 1, 0, 3)
    return out


def prep_shared(WKQ_re, WKQ_im, WPV_re, WPV_im):
    wr = np.ascontiguousarray(WKQ_re.T) * 256.0
    wi = np.ascontiguousarray(WKQ_im.T) * 256.0
    vr = np.ascontiguousarray(WPV_re.T) * 256.0
    vi = np.ascontiguousarray(WPV_im.T) * 256.0
    shared = {
        "wp_r": _pack_w(wr), "wp_i": _pack_w(wi),
        "wp_s": _pack_w(wr + wi),
        "vp_r": _pack(vr, False), "vp_i": _pack(vi, False),
        "vp_s": _pack(vr + vi, False),
        "trimask": np.triu(np.ones((P, P), np.float32)),
    }
    j = np.arange(T, dtype=np.float32)
    rho = 2.0 / np.maximum(j, 1.0)
    shared["rho2"] = np.ascontiguousarray(rho.reshape(TB, P).T)
    return shared


def kernel(E_re, E_im, WKQ_re, WKQ_im, WPV_re, WPV_im):
    E_re = np.asarray(E_re, dtype=np.float32)
    E_im = np.asarray(E_im, dtype=np.float32)
    shared = prep_shared(np.asarray(WKQ_re, np.float32),
                         np.asarray(WKQ_im, np.float32),
                         np.asarray(WPV_re, np.float32),
                         np.asarray(WPV_im, np.float32))
    in_maps = []
    for b in range(B):
        er = E_re[b] * 4.0
        ei = E_im[b] * 4.0
        m = dict(shared)
        m["ep_r"] = _pack(er, True)
        m["ep_i"] = _pack(ei, True)
        m["ep_ni"] = _pack(-ei, True)
        m["ep_s"] = _pack(er + ei, True)
        m["ep_d"] = _pack(er - ei, True)
        in_maps.append(m)

    nc = _get_module()
    res = run_bass_kernel_spmd(nc, in_maps, core_ids=list(range(B)))

    out = np.empty((B, D, T - 2), dtype=np.complex64)
    for b in range(B):
        r = res.results[b]["outT_re"]  # [T, D]
        i = res.results[b]["outT_im"]
        full = (r + 1j * i.astype(np.complex64)).T  # [D, T]
        out[b] = full[:, 1:T - 1]
    return out
